# revision 17
# baseline (speedup 1.0000x reference)
"""Trainium2 Bass kernel for multi-head attention with KV cache.

Problem: B=8, T_new=128, C=2048, H=16, D=128, T_past=2048.
Returns (out, K, V) like the reference:
    Q/K/V = x @ W* + b*  (split heads)
    K/V caches = concat(past, new)
    out = softmax(Q K^T / sqrt(D) + causal) V  -> @ Wo + bo

Sharding: tensor-parallel over heads, 2 heads per core x 8 cores.
Each core computes Q/K/V projections for its 2 heads (fp32r), runs
attention over all 8 batches (bf16 internals, fp32 accumulate), and
produces a partial output projection (bf16); host sums the 8 partials.

Projections all run d-major (out = W_chunk.T @ x^T chunk) as six
accumulation passes (Qh0 Kh0 Qh1 Kh1 Vd0 Vd1) of shape (128, B*T).
V is then PE-transposed per 128-block into the natural t-major layout
used by the attention O matmul (with a ones column appended so the
softmax denominators fall out of the same matmul).

Attention dataflow (per batch b, head h):
  S^T[t,q]  = sum_d KT[d,t] * QT[d,q]        (17 chunks of 128 t-rows)
  A^T       = exp(S^T * 1/sqrt(D) + mask)    (ACT engine, PSUM->SBUF bf16)
  O[q,0:128], O[q,128] = sum_t A^T[t,q] * [V[t,:] | 1]
  O_norm    = O[:, :128] * recip(O[:,128])
  O^T       = PE-transpose(O_norm)           -> out-projection lhsT
"""

import os
import sys

sys.path.insert(0, "/opt/trn_rl_repo")

import numpy as np
import ml_dtypes

import concourse.bass as bass
import concourse.tile as tile
from concourse import mybir, bacc
from concourse.bass_utils import run_bass_kernel_spmd

BF16 = ml_dtypes.bfloat16

B, T, C = 8, 128, 2048
H, TP, D = 16, 2048, 128
NCORES = 8
HPC = H // NCORES          # heads per core = 2
TT = TP + T                # total keys = 2176
NCH = TT // 128            # 17 chunks of keys
BT = B * T                 # 1024
KC = C // 128              # 16 contraction chunks
KVW = TP + 16 * 129        # packed K^T | V-chunks width = 4112
SCALE = 1.0 / float(np.sqrt(D))
NEG = -1.0e9

# module-level knobs for test.py
TRACE = False
LAST_EXEC_NS = None
LAST_RESULTS = None

_PROGRAM_CACHE = {}


def _chunk_classes(off):
    """Classify each key chunk: 'full' (all visible), 'part', 'skip'."""
    classes = []
    for c in range(NCH):
        if c < NCH - 1:
            kg0, kg1 = c * 128, c * 128 + 127
        else:
            kg0, kg1 = off, off + 127
        if kg1 <= off:                      # visible for every query
            classes.append("full")
        elif kg0 > off + 127:               # hidden for every query
            classes.append("skip")
        else:
            classes.append("part")
    return classes


def _mask_tile(off, c):
    """Additive fp32 mask (128 t x 128 q) for a 'part' chunk."""
    tt = np.arange(128)
    kg = (c * 128 + tt) if c < NCH - 1 else (off + tt)
    q = off + np.arange(128)
    vis = kg[:, None] <= q[None, :]
    return np.where(vis, np.float32(0.0), np.float32(NEG))


def _build_program(off):
    classes = _chunk_classes(off)
    part_chunks = [c for c in range(NCH) if classes[c] == "part"]
    live_chunks = [c for c in range(NCH) if classes[c] != "skip"]

    nc = bacc.Bacc("TRN2", target_bir_lowering=False, debug=False,
                   num_devices=NCORES)
    f32 = mybir.dt.float32
    f32r = mybir.dt.float32r
    bf16 = mybir.dt.bfloat16

    # ---- DRAM parameters (per-core data, host-packed layouts) ----
    p_xt = nc.declare_dram_parameter("xt", [128, KC * BT], f32r, isOutput=False)
    p_wq = [nc.declare_dram_parameter(f"wq{h}", [128, KC * 128], f32r, isOutput=False) for h in range(HPC)]
    p_wk = [nc.declare_dram_parameter(f"wk{h}", [128, KC * 128], f32r, isOutput=False) for h in range(HPC)]
    p_wv = nc.declare_dram_parameter("wv", [128, KC * 256], f32r, isOutput=False)
    p_wo = [nc.declare_dram_parameter(f"wo{h}", [128, C], bf16, isOutput=False) for h in range(HPC)]
    p_bq = [nc.declare_dram_parameter(f"bq{h}", [128, 1], f32, isOutput=False) for h in range(HPC)]
    p_bk = [nc.declare_dram_parameter(f"bk{h}", [128, 1], f32, isOutput=False) for h in range(HPC)]
    p_bv = [nc.declare_dram_parameter(f"bv{h}", [128, 1], f32, isOutput=False) for h in range(HPC)]
    # per (b,h): K^T (128 x 2048) and V packed with ones col (128 x 2064)
    p_kt = nc.declare_dram_parameter("kt", [B, HPC, 128, TP], bf16, isOutput=False)
    p_vp = nc.declare_dram_parameter("vp", [B, HPC, 128, 16 * 129], bf16, isOutput=False)
    p_id = nc.declare_dram_parameter("ident", [128, 128], bf16, isOutput=False)
    p_idf = nc.declare_dram_parameter("identf", [128, 128], f32, isOutput=False)
    p_mask = {c: nc.declare_dram_parameter(f"mask{c}", [128, 128], f32, isOutput=False) for c in part_chunks}

    o_out = nc.declare_dram_parameter("out_partial", [B, 128, C], f32, isOutput=True)
    o_kn = [nc.declare_dram_parameter(f"knew{h}", [128, BT], f32, isOutput=True) for h in range(HPC)]
    o_vn = [nc.declare_dram_parameter(f"vnewT{h}", [128, BT], f32, isOutput=True) for h in range(HPC)]

    with tile.TileContext(nc) as tc:
        with tc.tile_pool(name="sbA", bufs=1) as sbA, \
             tc.tile_pool(name="const", bufs=1) as constp, \
             tc.tile_pool(name="pers", bufs=1) as pers, \
             tc.tile_pool(name="ktstream", bufs=5) as ktpool, \
             tc.tile_pool(name="vpstream", bufs=5) as vppool, \
             tc.tile_pool(name="astream", bufs=3) as apool, \
             tc.tile_pool(name="outsb", bufs=2) as outsb:

            # --- phase A inputs first in trace order (weights, then x^T) ---
            wq_t, wk_t = [], []
            for h in range(HPC):
                w = sbA.tile([128, KC * 128], f32r, tag=f"wq{h}", name=f"wqt{h}")
                nc.sync.dma_start(w[:], p_wq[h][:])
                wq_t.append(w)
                w = sbA.tile([128, KC * 128], f32r, tag=f"wk{h}", name=f"wkt{h}")
                nc.sync.dma_start(w[:], p_wk[h][:])
                wk_t.append(w)
            xt = [sbA.tile([128, BT], f32r, tag=f"xt{k}", name=f"xt{k}") for k in range(KC)]
            for k in range(KC):
                nc.sync.dma_start(xt[k][:], p_xt[:, k * BT:(k + 1) * BT])
            wv_t = sbA.tile([128, KC * 256], f32r, tag="wv")
            nc.sync.dma_start(wv_t[:], p_wv[:])

            # --- small constants ---
            ident = constp.tile([128, 128], bf16, tag="ident")
            nc.sync.dma_start(ident[:], p_id[:])
            identf = constp.tile([128, 128], f32, tag="identf")
            nc.sync.dma_start(identf[:], p_idf[:])
            masks = {}
            for c in part_chunks:
                mt = constp.tile([128, 128], f32, tag=f"mask{c}", name=f"mask{c}")
                nc.sync.dma_start(mt[:], p_mask[c][:])
                masks[c] = mt
            bq_t, bk_t, bv_t = [], [], []
            for h in range(HPC):
                t1 = constp.tile([128, 1], f32, tag=f"bq{h}", name=f"bqt{h}")
                nc.sync.dma_start(t1[:], p_bq[h][:])
                bq_t.append(t1)
                t2 = constp.tile([128, 1], f32, tag=f"bk{h}", name=f"bkt{h}")
                nc.sync.dma_start(t2[:], p_bk[h][:])
                bk_t.append(t2)
                t3 = constp.tile([128, 1], f32, tag=f"bv{h}", name=f"bvt{h}")
                nc.sync.dma_start(t3[:], p_bv[h][:])
                bv_t.append(t3)
            wo_t = []
            for h in range(HPC):
                w = constp.tile([128, C], bf16, tag=f"wo{h}", name=f"wot{h}")
                nc.sync.dma_start(w[:], p_wo[h][:])
                wo_t.append(w)

            qT = [pers.tile([128, BT], bf16, tag=f"qT{h}", name=f"qT{h}") for h in range(HPC)]
            kTn = [pers.tile([128, BT], bf16, tag=f"kTn{h}", name=f"kTn{h}") for h in range(HPC)]
            vbf = [pers.tile([128, 258], bf16, tag=f"vbf{b}", name=f"vbf{b}") for b in range(B)]
            oT = [pers.tile([128, BT], bf16, tag=f"oT{h}", name=f"oT{h}") for h in range(HPC)]

            with tc.tile_pool(name="psA", bufs=1, space="PSUM") as psA, \
                 tc.tile_pool(name="psST", bufs=2, space="PSUM") as psST, \
                 tc.tile_pool(name="psO", bufs=2, space="PSUM") as psO, \
                 tc.tile_pool(name="psC", bufs=2, space="PSUM") as psC:

                def proj_pass(name, w_ap, n0, nw):
                    """One (128, BT) accumulation pass over the 16 c-chunks.
                    w_ap(k) yields the (128,128) stationary chunk."""
                    acc = psA.tile([128, BT], f32, tag="acc", name=f"acc_{name}")
                    for k in range(KC):
                        for nn in range(2):
                            nc.tensor.matmul(
                                acc[:, nn * 512:(nn + 1) * 512],
                                w_ap(k),
                                xt[k][:, nn * 512:(nn + 1) * 512],
                                start=(k == 0), stop=(k == KC - 1))
                    return acc

                def qk_pass(h):
                    acc = proj_pass(f"q{h}", lambda k, h=h: wq_t[h][:, k * 128:(k + 1) * 128], 0, 0)
                    nc.vector.tensor_scalar_add(qT[h][:], acc[:], bq_t[h][:])
                    acc = proj_pass(f"k{h}", lambda k, h=h: wk_t[h][:, k * 128:(k + 1) * 128], 0, 0)
                    nc.vector.tensor_scalar_add(kTn[h][:], acc[:], bk_t[h][:])
                    ksb = sbA.tile([128, BT], f32, tag="ksb", name=f"ksb{h}")
                    nc.vector.tensor_scalar_add(ksb[:], acc[:], bk_t[h][:])
                    nc.sync.dma_start(o_kn[h][:], ksb[:])

                def v_pass(h):
                    """V pass (d-major), then transpose into t-major vbf tiles."""
                    acc = proj_pass(f"v{h}", lambda k, h=h: wv_t[:, k * 256 + h * 128: k * 256 + (h + 1) * 128], 0, 0)
                    vsb = sbA.tile([128, BT], f32, tag="vsb", name=f"vsb{h}")
                    nc.vector.tensor_scalar_add(vsb[:], acc[:], bv_t[h][:])
                    nc.sync.dma_start(o_vn[h][:], vsb[:])
                    for b in range(B):
                        vtp = psC.tile([128, 128], f32, tag="cp", name=f"vtp{h}_{b}")
                        nc.tensor.transpose(vtp[:], vsb[:, b * 128:(b + 1) * 128], identf[:])
                        nc.scalar.copy(vbf[b][:, h * 129:h * 129 + 128], vtp[:])
                        nc.vector.memset(vbf[b][:, h * 129 + 128:h * 129 + 129], 1.0)

                qk_pass(0)
                v_pass(0)
                v_pass(1)
                qk_pass(1)

                # ---------------- attention (bf16) ----------------
                for h in range(HPC):
                    for b in range(B):
                        kt_t = ktpool.tile([128, TP], bf16, tag="kt", name=f"kt{h}_{b}")
                        nc.sync.dma_start(kt_t[:], p_kt[b, h])
                        at = apool.tile([128, NCH * 128], bf16, tag="at", name=f"at{h}_{b}")

                        # S^T in groups of 4 chunks per psum bank
                        for g in range(0, NCH, 4):
                            grp = [c for c in range(g, min(g + 4, NCH))
                                   if classes[c] != "skip"]
                            if not grp:
                                continue
                            sp = psST.tile([128, 512], f32, tag="sp", name=f"sp{h}_{b}_{g}")
                            for c in grp:
                                j = c - g
                                if c < NCH - 1:
                                    lhsT = kt_t[:, c * 128:(c + 1) * 128]
                                else:
                                    lhsT = kTn[h][:, b * 128:(b + 1) * 128]
                                nc.tensor.matmul(
                                    sp[:, j * 128:(j + 1) * 128], lhsT,
                                    qT[h][:, b * 128:(b + 1) * 128],
                                    start=True, stop=True)
                                if classes[c] == "part":
                                    nc.vector.tensor_add(
                                        sp[:, j * 128:(j + 1) * 128],
                                        sp[:, j * 128:(j + 1) * 128],
                                        masks[c][:])
                            j0, j1 = grp[0] - g, grp[-1] - g + 1
                            nc.scalar.activation(
                                at[:, (g + j0) * 128:(g + j1) * 128],
                                sp[:, j0 * 128:j1 * 128],
                                mybir.ActivationFunctionType.Exp,
                                scale=SCALE)

                        vp_t = vppool.tile([128, 16 * 129], bf16, tag="vp", name=f"vp{h}_{b}")
                        nc.sync.dma_start(vp_t[:], p_vp[b, h])
                        op = psO.tile([128, 129], f32, tag="op", name=f"op{h}_{b}")
                        for i, c in enumerate(live_chunks):
                            if c < NCH - 1:
                                rhs = vp_t[:, c * 129:(c + 1) * 129]
                            else:
                                rhs = vbf[b][:, h * 129:(h + 1) * 129]
                            nc.tensor.matmul(
                                op[:], at[:, c * 128:(c + 1) * 128], rhs,
                                start=(i == 0), stop=(i == len(live_chunks) - 1))

                        rec = outsb.tile([128, 1], f32, tag="rec", name=f"rec{h}_{b}")
                        nc.vector.reciprocal(rec[:], op[:, 128:129])
                        osb = outsb.tile([128, 128], bf16, tag="osb", name=f"osb{h}_{b}")
                        nc.vector.tensor_scalar_mul(osb[:], op[:, 0:128], rec[:])
                        otp = psST.tile([128, 128], bf16, tag="sp", name=f"otp{h}_{b}")
                        nc.tensor.transpose(otp[:], osb[:], ident[:])
                        nc.scalar.copy(oT[h][:, b * 128:(b + 1) * 128], otp[:])

                # ---------------- output projection (bf16) ----------------
                for b in range(B):
                    for half in range(2):
                        outt = outsb.tile([128, C // 2], f32, tag="outt",
                                          name=f"outt{b}_{half}", bufs=2)
                        for nn2 in range(2):
                            nn = half * 2 + nn2
                            cp = psC.tile([128, 512], f32, tag="cp", name=f"cp{b}_{nn}")
                            for h in range(HPC):
                                nc.tensor.matmul(
                                    cp[:], oT[h][:, b * 128:(b + 1) * 128],
                                    wo_t[h][:, nn * 512:(nn + 1) * 512],
                                    start=(h == 0), stop=(h == HPC - 1))
                            nc.vector.tensor_copy(outt[:, nn2 * 512:(nn2 + 1) * 512], cp[:])
                        nc.sync.dma_start(
                            o_out[b][:, half * (C // 2):(half + 1) * (C // 2)], outt[:])

    nc.compile()
    return nc


def _prep_inputs(x, K_past, V_past, Wq, bq, Wk, bk, Wv, bv, Wo, bo, off):
    """Build the 8 per-core input maps (host-side packing)."""
    x_flat = np.ascontiguousarray(x.reshape(BT, C)).astype(np.float32)
    xt = np.ascontiguousarray(
        x_flat.T.reshape(KC, 128, BT).transpose(1, 0, 2)).reshape(128, KC * BT)

    ident = np.eye(128, dtype=BF16)
    classes = _chunk_classes(off)
    mask_arrs = {c: _mask_tile(off, c) for c in range(NCH) if classes[c] == "part"}

    in_maps = []
    for core in range(NCORES):
        g0 = core * HPC
        im = {"xt": xt, "ident": ident, "identf": np.eye(128, dtype=np.float32)}
        for c, m in mask_arrs.items():
            im[f"mask{c}"] = m
        for h in range(HPC):
            g = g0 + h
            wq_h = Wq[:, g * D:(g + 1) * D]
            im[f"wq{h}"] = np.ascontiguousarray(
                wq_h.reshape(KC, 128, D).transpose(1, 0, 2)).reshape(128, KC * D)
            wk_h = Wk[:, g * D:(g + 1) * D]
            im[f"wk{h}"] = np.ascontiguousarray(
                wk_h.reshape(KC, 128, D).transpose(1, 0, 2)).reshape(128, KC * D)
            im[f"wo{h}"] = np.ascontiguousarray(Wo[g * D:(g + 1) * D, :]).astype(BF16)
            im[f"bq{h}"] = np.ascontiguousarray(bq[g * D:(g + 1) * D]).reshape(128, 1).astype(np.float32)
            im[f"bk{h}"] = np.ascontiguousarray(bk[g * D:(g + 1) * D]).reshape(128, 1).astype(np.float32)
            im[f"bv{h}"] = np.ascontiguousarray(bv[g * D:(g + 1) * D]).reshape(128, 1).astype(np.float32)
        wv_c = Wv[:, g0 * D:(g0 + HPC) * D]  # (2048, 256)
        im["wv"] = np.ascontiguousarray(
            wv_c.reshape(KC, 128, 256).transpose(1, 0, 2)).reshape(128, KC * 256)
        kp_c = K_past[:, g0:g0 + HPC]        # (B, 2, TP, D)
        im["kt"] = np.ascontiguousarray(kp_c.transpose(0, 1, 3, 2)).astype(BF16)
        vp_c = V_past[:, g0:g0 + HPC]
        vp_l = vp_c.reshape(B, HPC, 16, 128, D).transpose(0, 1, 3, 2, 4)  # (B,2,128,16,D)
        vp = np.empty((B, HPC, 128, 16, 129), dtype=BF16)
        vp[:, :, :, :, :D] = vp_l.astype(BF16)
        vp[:, :, :, :, D] = np.array(1.0, BF16)
        im["vp"] = vp.reshape(B, HPC, 128, 16 * 129)
        in_maps.append(im)
    return in_maps


def kernel(x, K_past, V_past, Wq, bq, Wk, bk, Wv, bv, Wo, bo, position_offset):
    global LAST_EXEC_NS, LAST_RESULTS
    off = int(position_offset)
    x = np.asarray(x, dtype=np.float32)
    K_past = np.asarray(K_past, dtype=np.float32)
    V_past = np.asarray(V_past, dtype=np.float32)
    Wq, Wk, Wv, Wo = (np.asarray(a, dtype=np.float32) for a in (Wq, Wk, Wv, Wo))
    bq, bk, bv, bo = (np.asarray(a, dtype=np.float32) for a in (bq, bk, bv, bo))

    if off not in _PROGRAM_CACHE:
        _PROGRAM_CACHE[off] = _build_program(off)
    nc = _PROGRAM_CACHE[off]

    in_maps = _prep_inputs(x, K_past, V_past, Wq, bq, Wk, bk, Wv, bv, Wo, bo, off)
    res = run_bass_kernel_spmd(nc, in_maps, list(range(NCORES)), trace=TRACE)
    LAST_EXEC_NS = res.exec_time_ns
    LAST_RESULTS = res

    # ---- host assembly ----
    out = np.zeros((BT, C), np.float32)
    for core in range(NCORES):
        out += res.results[core]["out_partial"].reshape(BT, C)
    out += bo[None, :]
    out = out.reshape(B, T, C)

    K_full = np.empty((B, H, TT, D), np.float32)
    V_full = np.empty((B, H, TT, D), np.float32)
    K_full[:, :, :TP] = K_past
    V_full[:, :, :TP] = V_past
    for core in range(NCORES):
        for h in range(HPC):
            g = core * HPC + h
            kn = res.results[core][f"knew{h}"]          # (128 d, BT)
            K_full[:, g, TP:] = kn.reshape(D, B, T).transpose(1, 2, 0)
            vn = res.results[core][f"vnewT{h}"]         # (128 d, BT)
            V_full[:, g, TP:] = vn.reshape(D, B, T).transpose(1, 2, 0)
    return out, K_full, V_full


# revision 24
# speedup vs baseline: 1.1091x; 1.1091x over previous
"""Trainium2 Bass kernel for multi-head attention with KV cache.

Problem: B=8, T_new=128, C=2048, H=16, D=128, T_past=2048.
Returns (out, K, V) like the reference:
    Q/K/V = x @ W* + b*  (split heads)
    K/V caches = concat(past, new)
    out = softmax(Q K^T / sqrt(D) + causal) V  -> @ Wo + bo

Sharding: tensor-parallel over heads, 2 heads per core x 8 cores.
Each core computes Q/K/V projections for its 2 heads (fp32r), runs
attention over all 8 batches (bf16 internals, fp32 accumulate), and
produces a partial output projection (bf16); host sums the 8 partials.

Projections all run d-major (out = W_chunk.T @ x^T chunk) as six
accumulation passes (Qh0 Kh0 Qh1 Kh1 Vd0 Vd1) of shape (128, B*T).
V is then PE-transposed per 128-block into the natural t-major layout
used by the attention O matmul (with a ones column appended so the
softmax denominators fall out of the same matmul).

Attention dataflow (per batch b, head h):
  S^T[t,q]  = sum_d KT[d,t] * QT[d,q]        (17 chunks of 128 t-rows)
  A^T       = exp(S^T * 1/sqrt(D) + mask)    (ACT engine, PSUM->SBUF bf16)
  O[q,0:128], O[q,128] = sum_t A^T[t,q] * [V[t,:] | 1]
  O_norm    = O[:, :128] * recip(O[:,128])
  O^T       = PE-transpose(O_norm)           -> out-projection lhsT
"""

import os
import sys

sys.path.insert(0, "/opt/trn_rl_repo")

import numpy as np
import ml_dtypes

import concourse.bass as bass
import concourse.tile as tile
from concourse import mybir, bacc
from concourse.bass_utils import run_bass_kernel_spmd

BF16 = ml_dtypes.bfloat16

B, T, C = 8, 128, 2048
H, TP, D = 16, 2048, 128
NCORES = 8
HPC = H // NCORES          # heads per core = 2
TT = TP + T                # total keys = 2176
NCH = TT // 128            # 17 chunks of keys
BT = B * T                 # 1024
KC = C // 128              # 16 contraction chunks
KVW = TP + 16 * 129        # packed K^T | V-chunks width = 4112
SCALE = 1.0 / float(np.sqrt(D))
NEG = -1.0e9

# module-level knobs for test.py
TRACE = False
LAST_EXEC_NS = None
LAST_RESULTS = None
PROJ_BF16 = True   # bf16 projections (faster); False = fp32r (K/V ~1.3e-4)

_PROGRAM_CACHE = {}


def _chunk_classes(off):
    """Classify each key chunk: 'full' (all visible), 'part', 'skip'."""
    classes = []
    for c in range(NCH):
        if c < NCH - 1:
            kg0, kg1 = c * 128, c * 128 + 127
        else:
            kg0, kg1 = off, off + 127
        if kg1 <= off:                      # visible for every query
            classes.append("full")
        elif kg0 > off + 127:               # hidden for every query
            classes.append("skip")
        else:
            classes.append("part")
    return classes


def _mask_tile(off, c):
    """Additive fp32 mask (128 t x 128 q) for a 'part' chunk."""
    tt = np.arange(128)
    kg = (c * 128 + tt) if c < NCH - 1 else (off + tt)
    q = off + np.arange(128)
    vis = kg[:, None] <= q[None, :]
    return np.where(vis, np.float32(0.0), np.float32(NEG))


def _build_program(off):
    classes = _chunk_classes(off)
    part_chunks = [c for c in range(NCH) if classes[c] == "part"]
    live_chunks = [c for c in range(NCH) if classes[c] != "skip"]

    nc = bacc.Bacc("TRN2", target_bir_lowering=False, debug=False,
                   num_devices=NCORES)
    f32 = mybir.dt.float32
    f32r = mybir.dt.bfloat16 if PROJ_BF16 else mybir.dt.float32r
    bf16 = mybir.dt.bfloat16

    # ---- DRAM parameters (per-core data, host-packed layouts) ----
    p_xt = nc.declare_dram_parameter("xt", [128, KC * BT], f32r, isOutput=False)
    p_wq = [nc.declare_dram_parameter(f"wq{h}", [128, KC * 128], f32r, isOutput=False) for h in range(HPC)]
    p_wk = [nc.declare_dram_parameter(f"wk{h}", [128, KC * 128], f32r, isOutput=False) for h in range(HPC)]
    p_wv = nc.declare_dram_parameter("wv", [128, KC * 256], f32r, isOutput=False)
    p_wo = [nc.declare_dram_parameter(f"wo{h}", [128, C], bf16, isOutput=False) for h in range(HPC)]
    p_bq = [nc.declare_dram_parameter(f"bq{h}", [128, 1], f32, isOutput=False) for h in range(HPC)]
    p_bk = [nc.declare_dram_parameter(f"bk{h}", [128, 1], f32, isOutput=False) for h in range(HPC)]
    p_bv = [nc.declare_dram_parameter(f"bv{h}", [128, 1], f32, isOutput=False) for h in range(HPC)]
    # per (b,h): K^T (128 x 2048) and V packed with ones col (128 x 2064)
    p_kt = nc.declare_dram_parameter("kt", [B, HPC, 128, TP], bf16, isOutput=False)
    p_vp = nc.declare_dram_parameter("vp", [B, HPC, 128, 16 * 129], bf16, isOutput=False)
    p_id = nc.declare_dram_parameter("ident", [128, 128], bf16, isOutput=False)
    p_idf = nc.declare_dram_parameter("identf", [128, 128], f32, isOutput=False)
    p_mask = {c: nc.declare_dram_parameter(f"mask{c}", [128, 128], f32, isOutput=False) for c in part_chunks}

    o_out = nc.declare_dram_parameter("out_partial", [B, 128, C], f32, isOutput=True)
    o_kn = [nc.declare_dram_parameter(f"knew{h}", [128, BT], f32, isOutput=True) for h in range(HPC)]
    o_vn = [nc.declare_dram_parameter(f"vnewT{h}", [128, BT], f32, isOutput=True) for h in range(HPC)]

    with tile.TileContext(nc) as tc:
        with tc.tile_pool(name="sbA", bufs=1) as sbA, \
             tc.tile_pool(name="const", bufs=1) as constp, \
             tc.tile_pool(name="pers", bufs=1) as pers, \
             tc.tile_pool(name="ktstream", bufs=5) as ktpool, \
             tc.tile_pool(name="vpstream", bufs=5) as vppool, \
             tc.tile_pool(name="astream", bufs=3) as apool, \
             tc.tile_pool(name="outsb", bufs=2) as outsb:

            # --- phase A inputs first in trace order (weights, then x^T) ---
            wq_t, wk_t = [], []
            for h in range(HPC):
                w = sbA.tile([128, KC * 128], f32r, tag=f"wq{h}", name=f"wqt{h}")
                nc.sync.dma_start(w[:], p_wq[h][:])
                wq_t.append(w)
                w = sbA.tile([128, KC * 128], f32r, tag=f"wk{h}", name=f"wkt{h}")
                nc.sync.dma_start(w[:], p_wk[h][:])
                wk_t.append(w)
            xt = [sbA.tile([128, BT], f32r, tag=f"xt{k}", name=f"xt{k}") for k in range(KC)]
            for k in range(KC):
                nc.sync.dma_start(xt[k][:], p_xt[:, k * BT:(k + 1) * BT])
            wv_t = sbA.tile([128, KC * 256], f32r, tag="wv")
            nc.sync.dma_start(wv_t[:], p_wv[:])

            # --- small constants ---
            ident = constp.tile([128, 128], bf16, tag="ident")
            nc.sync.dma_start(ident[:], p_id[:])
            identf = constp.tile([128, 128], f32, tag="identf")
            nc.sync.dma_start(identf[:], p_idf[:])
            masks = {}
            for c in part_chunks:
                mt = constp.tile([128, 128], f32, tag=f"mask{c}", name=f"mask{c}")
                nc.sync.dma_start(mt[:], p_mask[c][:])
                masks[c] = mt
            bq_t, bk_t, bv_t = [], [], []
            for h in range(HPC):
                t1 = constp.tile([128, 1], f32, tag=f"bq{h}", name=f"bqt{h}")
                nc.sync.dma_start(t1[:], p_bq[h][:])
                bq_t.append(t1)
                t2 = constp.tile([128, 1], f32, tag=f"bk{h}", name=f"bkt{h}")
                nc.sync.dma_start(t2[:], p_bk[h][:])
                bk_t.append(t2)
                t3 = constp.tile([128, 1], f32, tag=f"bv{h}", name=f"bvt{h}")
                nc.sync.dma_start(t3[:], p_bv[h][:])
                bv_t.append(t3)
            wo_t = []
            for h in range(HPC):
                w = constp.tile([128, C], bf16, tag=f"wo{h}", name=f"wot{h}")
                nc.sync.dma_start(w[:], p_wo[h][:])
                wo_t.append(w)

            qT = [pers.tile([128, BT], bf16, tag=f"qT{h}", name=f"qT{h}") for h in range(HPC)]
            kTn = [pers.tile([128, BT], bf16, tag=f"kTn{h}", name=f"kTn{h}") for h in range(HPC)]
            vbf = [pers.tile([128, 258], bf16, tag=f"vbf{b}", name=f"vbf{b}") for b in range(B)]
            oT = [pers.tile([128, BT], bf16, tag=f"oT{h}", name=f"oT{h}") for h in range(HPC)]

            with tc.tile_pool(name="psA", bufs=1, space="PSUM") as psA, \
                 tc.tile_pool(name="psST", bufs=2, space="PSUM") as psST, \
                 tc.tile_pool(name="psO", bufs=2, space="PSUM") as psO, \
                 tc.tile_pool(name="psC", bufs=2, space="PSUM") as psC:

                def proj_pass(name, w_ap, n0, nw):
                    """One (128, BT) accumulation pass over the 16 c-chunks.
                    w_ap(k) yields the (128,128) stationary chunk."""
                    acc = psA.tile([128, BT], f32, tag="acc", name=f"acc_{name}")
                    for k in range(KC):
                        for nn in range(2):
                            nc.tensor.matmul(
                                acc[:, nn * 512:(nn + 1) * 512],
                                w_ap(k),
                                xt[k][:, nn * 512:(nn + 1) * 512],
                                start=(k == 0), stop=(k == KC - 1))
                    return acc

                def qk_pass(h):
                    acc = proj_pass(f"q{h}", lambda k, h=h: wq_t[h][:, k * 128:(k + 1) * 128], 0, 0)
                    nc.vector.tensor_scalar_add(qT[h][:], acc[:], bq_t[h][:])
                    acc = proj_pass(f"k{h}", lambda k, h=h: wk_t[h][:, k * 128:(k + 1) * 128], 0, 0)
                    nc.vector.tensor_scalar_add(kTn[h][:], acc[:], bk_t[h][:])
                    ksb = sbA.tile([128, BT], f32, tag="ksb", name=f"ksb{h}")
                    nc.vector.tensor_scalar_add(ksb[:], acc[:], bk_t[h][:])
                    nc.sync.dma_start(o_kn[h][:], ksb[:])

                def v_pass(h):
                    """V pass (d-major), then transpose into t-major vbf tiles."""
                    acc = proj_pass(f"v{h}", lambda k, h=h: wv_t[:, k * 256 + h * 128: k * 256 + (h + 1) * 128], 0, 0)
                    vsb = sbA.tile([128, BT], f32, tag="vsb", name=f"vsb{h}")
                    nc.vector.tensor_scalar_add(vsb[:], acc[:], bv_t[h][:])
                    nc.sync.dma_start(o_vn[h][:], vsb[:])
                    for b in range(B):
                        vtp = psC.tile([128, 128], f32, tag="cp", name=f"vtp{h}_{b}")
                        nc.tensor.transpose(vtp[:], vsb[:, b * 128:(b + 1) * 128], identf[:])
                        nc.scalar.copy(vbf[b][:, h * 129:h * 129 + 128], vtp[:])
                        nc.vector.memset(vbf[b][:, h * 129 + 128:h * 129 + 129], 1.0)

                qk_pass(0)
                v_pass(0)
                qk_pass(1)
                v_pass(1)

                # ---------------- attention (bf16) ----------------
                for h in range(HPC):
                    for b in range(B):
                        kt_t = ktpool.tile([128, TP], bf16, tag="kt", name=f"kt{h}_{b}")
                        nc.sync.dma_start(kt_t[:], p_kt[b, h])
                        at = apool.tile([128, NCH * 128], bf16, tag="at", name=f"at{h}_{b}")

                        # S^T in groups of 4 chunks per psum bank
                        for g in range(0, NCH, 4):
                            grp = [c for c in range(g, min(g + 4, NCH))
                                   if classes[c] != "skip"]
                            if not grp:
                                continue
                            sp = psST.tile([128, 512], f32, tag="sp", name=f"sp{h}_{b}_{g}")
                            for c in grp:
                                j = c - g
                                if c < NCH - 1:
                                    lhsT = kt_t[:, c * 128:(c + 1) * 128]
                                else:
                                    lhsT = kTn[h][:, b * 128:(b + 1) * 128]
                                nc.tensor.matmul(
                                    sp[:, j * 128:(j + 1) * 128], lhsT,
                                    qT[h][:, b * 128:(b + 1) * 128],
                                    start=True, stop=True)
                                if classes[c] == "part":
                                    nc.vector.tensor_add(
                                        sp[:, j * 128:(j + 1) * 128],
                                        sp[:, j * 128:(j + 1) * 128],
                                        masks[c][:])
                            j0, j1 = grp[0] - g, grp[-1] - g + 1
                            nc.scalar.activation(
                                at[:, (g + j0) * 128:(g + j1) * 128],
                                sp[:, j0 * 128:j1 * 128],
                                mybir.ActivationFunctionType.Exp,
                                scale=SCALE)

                        vp_t = vppool.tile([128, 16 * 129], bf16, tag="vp", name=f"vp{h}_{b}")
                        nc.sync.dma_start(vp_t[:], p_vp[b, h])
                        op = psO.tile([128, 129], f32, tag="op", name=f"op{h}_{b}")
                        for i, c in enumerate(live_chunks):
                            if c < NCH - 1:
                                rhs = vp_t[:, c * 129:(c + 1) * 129]
                            else:
                                rhs = vbf[b][:, h * 129:(h + 1) * 129]
                            nc.tensor.matmul(
                                op[:], at[:, c * 128:(c + 1) * 128], rhs,
                                start=(i == 0), stop=(i == len(live_chunks) - 1))

                        rec = outsb.tile([128, 1], f32, tag="rec", name=f"rec{h}_{b}")
                        nc.vector.reciprocal(rec[:], op[:, 128:129])
                        osb = outsb.tile([128, 128], bf16, tag="osb", name=f"osb{h}_{b}")
                        nc.vector.tensor_scalar_mul(osb[:], op[:, 0:128], rec[:])
                        otp = psST.tile([128, 128], bf16, tag="sp", name=f"otp{h}_{b}")
                        nc.tensor.transpose(otp[:], osb[:], ident[:])
                        nc.scalar.copy(oT[h][:, b * 128:(b + 1) * 128], otp[:])

                # ---------------- output projection (bf16) ----------------
                for b in range(B):
                    for half in range(2):
                        outt = outsb.tile([128, C // 2], f32, tag="outt",
                                          name=f"outt{b}_{half}", bufs=2)
                        for nn2 in range(2):
                            nn = half * 2 + nn2
                            cp = psC.tile([128, 512], f32, tag="cp", name=f"cp{b}_{nn}")
                            for h in range(HPC):
                                nc.tensor.matmul(
                                    cp[:], oT[h][:, b * 128:(b + 1) * 128],
                                    wo_t[h][:, nn * 512:(nn + 1) * 512],
                                    start=(h == 0), stop=(h == HPC - 1))
                            nc.vector.tensor_copy(outt[:, nn2 * 512:(nn2 + 1) * 512], cp[:])
                        nc.sync.dma_start(
                            o_out[b][:, half * (C // 2):(half + 1) * (C // 2)], outt[:])

    nc.compile()
    return nc


def _prep_inputs(x, K_past, V_past, Wq, bq, Wk, bk, Wv, bv, Wo, bo, off):
    """Build the 8 per-core input maps (host-side packing)."""
    proj_dt = BF16 if PROJ_BF16 else np.float32
    x_flat = np.ascontiguousarray(x.reshape(BT, C)).astype(np.float32)
    xt = np.ascontiguousarray(
        x_flat.T.reshape(KC, 128, BT).transpose(1, 0, 2)).reshape(128, KC * BT).astype(proj_dt)

    ident = np.eye(128, dtype=BF16)
    classes = _chunk_classes(off)
    mask_arrs = {c: _mask_tile(off, c) for c in range(NCH) if classes[c] == "part"}

    in_maps = []
    for core in range(NCORES):
        g0 = core * HPC
        im = {"xt": xt, "ident": ident, "identf": np.eye(128, dtype=np.float32)}
        for c, m in mask_arrs.items():
            im[f"mask{c}"] = m
        for h in range(HPC):
            g = g0 + h
            wq_h = Wq[:, g * D:(g + 1) * D]
            im[f"wq{h}"] = np.ascontiguousarray(
                wq_h.reshape(KC, 128, D).transpose(1, 0, 2)).reshape(128, KC * D).astype(proj_dt)
            wk_h = Wk[:, g * D:(g + 1) * D]
            im[f"wk{h}"] = np.ascontiguousarray(
                wk_h.reshape(KC, 128, D).transpose(1, 0, 2)).reshape(128, KC * D).astype(proj_dt)
            im[f"wo{h}"] = np.ascontiguousarray(Wo[g * D:(g + 1) * D, :]).astype(BF16)
            im[f"bq{h}"] = np.ascontiguousarray(bq[g * D:(g + 1) * D]).reshape(128, 1).astype(np.float32)
            im[f"bk{h}"] = np.ascontiguousarray(bk[g * D:(g + 1) * D]).reshape(128, 1).astype(np.float32)
            im[f"bv{h}"] = np.ascontiguousarray(bv[g * D:(g + 1) * D]).reshape(128, 1).astype(np.float32)
        wv_c = Wv[:, g0 * D:(g0 + HPC) * D]  # (2048, 256)
        im["wv"] = np.ascontiguousarray(
            wv_c.reshape(KC, 128, 256).transpose(1, 0, 2)).reshape(128, KC * 256).astype(proj_dt)
        kp_c = K_past[:, g0:g0 + HPC]        # (B, 2, TP, D)
        im["kt"] = np.ascontiguousarray(kp_c.transpose(0, 1, 3, 2)).astype(BF16)
        vp_c = V_past[:, g0:g0 + HPC]
        vp_l = vp_c.reshape(B, HPC, 16, 128, D).transpose(0, 1, 3, 2, 4)  # (B,2,128,16,D)
        vp = np.empty((B, HPC, 128, 16, 129), dtype=BF16)
        vp[:, :, :, :, :D] = vp_l.astype(BF16)
        vp[:, :, :, :, D] = np.array(1.0, BF16)
        im["vp"] = vp.reshape(B, HPC, 128, 16 * 129)
        in_maps.append(im)
    return in_maps


def kernel(x, K_past, V_past, Wq, bq, Wk, bk, Wv, bv, Wo, bo, position_offset):
    global LAST_EXEC_NS, LAST_RESULTS
    off = int(position_offset)
    x = np.asarray(x, dtype=np.float32)
    K_past = np.asarray(K_past, dtype=np.float32)
    V_past = np.asarray(V_past, dtype=np.float32)
    Wq, Wk, Wv, Wo = (np.asarray(a, dtype=np.float32) for a in (Wq, Wk, Wv, Wo))
    bq, bk, bv, bo = (np.asarray(a, dtype=np.float32) for a in (bq, bk, bv, bo))

    cache_key = (off, PROJ_BF16)
    if cache_key not in _PROGRAM_CACHE:
        _PROGRAM_CACHE[cache_key] = _build_program(off)
    nc = _PROGRAM_CACHE[cache_key]

    in_maps = _prep_inputs(x, K_past, V_past, Wq, bq, Wk, bk, Wv, bv, Wo, bo, off)
    res = run_bass_kernel_spmd(nc, in_maps, list(range(NCORES)), trace=TRACE)
    LAST_EXEC_NS = res.exec_time_ns
    LAST_RESULTS = res

    # ---- host assembly ----
    out = np.zeros((BT, C), np.float32)
    for core in range(NCORES):
        out += res.results[core]["out_partial"].reshape(BT, C)
    out += bo[None, :]
    out = out.reshape(B, T, C)

    K_full = np.empty((B, H, TT, D), np.float32)
    V_full = np.empty((B, H, TT, D), np.float32)
    K_full[:, :, :TP] = K_past
    V_full[:, :, :TP] = V_past
    for core in range(NCORES):
        for h in range(HPC):
            g = core * HPC + h
            kn = res.results[core][f"knew{h}"]          # (128 d, BT)
            K_full[:, g, TP:] = kn.reshape(D, B, T).transpose(1, 2, 0)
            vn = res.results[core][f"vnewT{h}"]         # (128 d, BT)
            V_full[:, g, TP:] = vn.reshape(D, B, T).transpose(1, 2, 0)
    return out, K_full, V_full


# revision 25
# speedup vs baseline: 1.1419x; 1.0295x over previous
"""Trainium2 Bass kernel for multi-head attention with KV cache.

Problem: B=8, T_new=128, C=2048, H=16, D=128, T_past=2048.
Returns (out, K, V) like the reference:
    Q/K/V = x @ W* + b*  (split heads)
    K/V caches = concat(past, new)
    out = softmax(Q K^T / sqrt(D) + causal) V  -> @ Wo + bo

Sharding: tensor-parallel over heads, 2 heads per core x 8 cores.
Each core computes Q/K/V projections for its 2 heads (fp32r), runs
attention over all 8 batches (bf16 internals, fp32 accumulate), and
produces a partial output projection (bf16); host sums the 8 partials.

Projections all run d-major (out = W_chunk.T @ x^T chunk) as six
accumulation passes (Qh0 Kh0 Qh1 Kh1 Vd0 Vd1) of shape (128, B*T).
V is then PE-transposed per 128-block into the natural t-major layout
used by the attention O matmul (with a ones column appended so the
softmax denominators fall out of the same matmul).

Attention dataflow (per batch b, head h):
  S^T[t,q]  = sum_d KT[d,t] * QT[d,q]        (17 chunks of 128 t-rows)
  A^T       = exp(S^T * 1/sqrt(D) + mask)    (ACT engine, PSUM->SBUF bf16)
  O[q,0:128], O[q,128] = sum_t A^T[t,q] * [V[t,:] | 1]
  O_norm    = O[:, :128] * recip(O[:,128])
  O^T       = PE-transpose(O_norm)           -> out-projection lhsT
"""

import os
import sys

sys.path.insert(0, "/opt/trn_rl_repo")

import numpy as np
import ml_dtypes

import concourse.bass as bass
import concourse.tile as tile
from concourse import mybir, bacc
from concourse.bass_utils import run_bass_kernel_spmd

BF16 = ml_dtypes.bfloat16

B, T, C = 8, 128, 2048
H, TP, D = 16, 2048, 128
NCORES = 8
HPC = H // NCORES          # heads per core = 2
TT = TP + T                # total keys = 2176
NCH = TT // 128            # 17 chunks of keys
BT = B * T                 # 1024
KC = C // 128              # 16 contraction chunks
KVW = TP + 16 * 129        # packed K^T | V-chunks width = 4112
SCALE = 1.0 / float(np.sqrt(D))
NEG = -1.0e9

# module-level knobs for test.py
TRACE = False
LAST_EXEC_NS = None
LAST_RESULTS = None
PROJ_BF16 = True   # bf16 projections (faster); False = fp32r (K/V ~1.3e-4)

_PROGRAM_CACHE = {}


def _chunk_classes(off):
    """Classify each key chunk: 'full' (all visible), 'part', 'skip'."""
    classes = []
    for c in range(NCH):
        if c < NCH - 1:
            kg0, kg1 = c * 128, c * 128 + 127
        else:
            kg0, kg1 = off, off + 127
        if kg1 <= off:                      # visible for every query
            classes.append("full")
        elif kg0 > off + 127:               # hidden for every query
            classes.append("skip")
        else:
            classes.append("part")
    return classes


def _mask_tile(off, c):
    """Additive fp32 mask (128 t x 128 q) for a 'part' chunk."""
    tt = np.arange(128)
    kg = (c * 128 + tt) if c < NCH - 1 else (off + tt)
    q = off + np.arange(128)
    vis = kg[:, None] <= q[None, :]
    return np.where(vis, np.float32(1.0), np.float32(0.0)).astype(BF16)


def _build_program(off):
    classes = _chunk_classes(off)
    part_chunks = [c for c in range(NCH) if classes[c] == "part"]
    live_chunks = [c for c in range(NCH) if classes[c] != "skip"]

    nc = bacc.Bacc("TRN2", target_bir_lowering=False, debug=False,
                   num_devices=NCORES)
    f32 = mybir.dt.float32
    f32r = mybir.dt.bfloat16 if PROJ_BF16 else mybir.dt.float32r
    bf16 = mybir.dt.bfloat16

    # ---- DRAM parameters (per-core data, host-packed layouts) ----
    p_xt = nc.declare_dram_parameter("xt", [128, KC * BT], f32r, isOutput=False)
    p_wq = [nc.declare_dram_parameter(f"wq{h}", [128, KC * 128], f32r, isOutput=False) for h in range(HPC)]
    p_wk = [nc.declare_dram_parameter(f"wk{h}", [128, KC * 128], f32r, isOutput=False) for h in range(HPC)]
    p_wv = nc.declare_dram_parameter("wv", [128, KC * 256], f32r, isOutput=False)
    p_wo = [nc.declare_dram_parameter(f"wo{h}", [128, C], bf16, isOutput=False) for h in range(HPC)]
    p_bq = [nc.declare_dram_parameter(f"bq{h}", [128, 1], f32, isOutput=False) for h in range(HPC)]
    p_bk = [nc.declare_dram_parameter(f"bk{h}", [128, 1], f32, isOutput=False) for h in range(HPC)]
    p_bv = [nc.declare_dram_parameter(f"bv{h}", [128, 1], f32, isOutput=False) for h in range(HPC)]
    # per (b,h): K^T (128 x 2048) and V packed with ones col (128 x 2064)
    p_kt = nc.declare_dram_parameter("kt", [B, HPC, 128, TP], bf16, isOutput=False)
    p_vp = nc.declare_dram_parameter("vp", [B, HPC, 128, 16 * 129], bf16, isOutput=False)
    p_id = nc.declare_dram_parameter("ident", [128, 128], bf16, isOutput=False)
    p_idf = nc.declare_dram_parameter("identf", [128, 128], f32, isOutput=False)
    p_mask = {c: nc.declare_dram_parameter(f"mask{c}", [128, 128], bf16, isOutput=False) for c in part_chunks}

    o_out = nc.declare_dram_parameter("out_partial", [B, 128, C], f32, isOutput=True)
    o_kn = [nc.declare_dram_parameter(f"knew{h}", [128, BT], f32, isOutput=True) for h in range(HPC)]
    o_vn = [nc.declare_dram_parameter(f"vnewT{h}", [128, BT], f32, isOutput=True) for h in range(HPC)]

    with tile.TileContext(nc) as tc:
        with tc.tile_pool(name="sbA", bufs=1) as sbA, \
             tc.tile_pool(name="const", bufs=1) as constp, \
             tc.tile_pool(name="pers", bufs=1) as pers, \
             tc.tile_pool(name="ktstream", bufs=8) as ktpool, \
             tc.tile_pool(name="vpstream", bufs=8) as vppool, \
             tc.tile_pool(name="astream", bufs=4) as apool, \
             tc.tile_pool(name="outsb", bufs=2) as outsb:

            # --- phase A inputs first in trace order (weights, then x^T) ---
            wq_t, wk_t = [], []
            for h in range(HPC):
                w = sbA.tile([128, KC * 128], f32r, tag=f"wq{h}", name=f"wqt{h}")
                nc.sync.dma_start(w[:], p_wq[h][:])
                wq_t.append(w)
                w = sbA.tile([128, KC * 128], f32r, tag=f"wk{h}", name=f"wkt{h}")
                nc.sync.dma_start(w[:], p_wk[h][:])
                wk_t.append(w)
            xt = [sbA.tile([128, BT], f32r, tag=f"xt{k}", name=f"xt{k}") for k in range(KC)]
            for k in range(KC):
                nc.sync.dma_start(xt[k][:], p_xt[:, k * BT:(k + 1) * BT])
            wv_t = sbA.tile([128, KC * 256], f32r, tag="wv")
            nc.sync.dma_start(wv_t[:], p_wv[:])

            # --- small constants ---
            ident = constp.tile([128, 128], bf16, tag="ident")
            nc.sync.dma_start(ident[:], p_id[:])
            identf = constp.tile([128, 128], f32, tag="identf")
            nc.sync.dma_start(identf[:], p_idf[:])
            masks = {}
            for c in part_chunks:
                mt = constp.tile([128, 128], bf16, tag=f"mask{c}", name=f"mask{c}")
                nc.sync.dma_start(mt[:], p_mask[c][:])
                masks[c] = mt
            bq_t, bk_t, bv_t = [], [], []
            for h in range(HPC):
                t1 = constp.tile([128, 1], f32, tag=f"bq{h}", name=f"bqt{h}")
                nc.sync.dma_start(t1[:], p_bq[h][:])
                bq_t.append(t1)
                t2 = constp.tile([128, 1], f32, tag=f"bk{h}", name=f"bkt{h}")
                nc.sync.dma_start(t2[:], p_bk[h][:])
                bk_t.append(t2)
                t3 = constp.tile([128, 1], f32, tag=f"bv{h}", name=f"bvt{h}")
                nc.sync.dma_start(t3[:], p_bv[h][:])
                bv_t.append(t3)
            wo_t = []
            for h in range(HPC):
                w = constp.tile([128, C], bf16, tag=f"wo{h}", name=f"wot{h}")
                nc.sync.dma_start(w[:], p_wo[h][:])
                wo_t.append(w)

            qT = [pers.tile([128, BT], bf16, tag=f"qT{h}", name=f"qT{h}") for h in range(HPC)]
            kTn = [pers.tile([128, BT], bf16, tag=f"kTn{h}", name=f"kTn{h}") for h in range(HPC)]
            vbf = [pers.tile([128, 258], bf16, tag=f"vbf{b}", name=f"vbf{b}") for b in range(B)]
            oT = [pers.tile([128, BT], bf16, tag=f"oT{h}", name=f"oT{h}") for h in range(HPC)]

            with tc.tile_pool(name="psA", bufs=1, space="PSUM") as psA, \
                 tc.tile_pool(name="psST", bufs=3, space="PSUM") as psST, \
                 tc.tile_pool(name="psO", bufs=2, space="PSUM") as psO, \
                 tc.tile_pool(name="psC", bufs=1, space="PSUM") as psC:

                def proj_pass(name, w_ap, n0, nw):
                    """One (128, BT) accumulation pass over the 16 c-chunks.
                    w_ap(k) yields the (128,128) stationary chunk."""
                    acc = psA.tile([128, BT], f32, tag="acc", name=f"acc_{name}")
                    for k in range(KC):
                        for nn in range(2):
                            nc.tensor.matmul(
                                acc[:, nn * 512:(nn + 1) * 512],
                                w_ap(k),
                                xt[k][:, nn * 512:(nn + 1) * 512],
                                start=(k == 0), stop=(k == KC - 1))
                    return acc

                def qk_pass(h):
                    acc = proj_pass(f"q{h}", lambda k, h=h: wq_t[h][:, k * 128:(k + 1) * 128], 0, 0)
                    nc.vector.tensor_scalar_add(qT[h][:], acc[:], bq_t[h][:])
                    acc = proj_pass(f"k{h}", lambda k, h=h: wk_t[h][:, k * 128:(k + 1) * 128], 0, 0)
                    nc.vector.tensor_scalar_add(kTn[h][:], acc[:], bk_t[h][:])
                    ksb = sbA.tile([128, BT], f32, tag="ksb", name=f"ksb{h}")
                    nc.vector.tensor_scalar_add(ksb[:], acc[:], bk_t[h][:])
                    nc.sync.dma_start(o_kn[h][:], ksb[:])

                def v_pass(h):
                    """V pass (d-major), then transpose into t-major vbf tiles."""
                    acc = proj_pass(f"v{h}", lambda k, h=h: wv_t[:, k * 256 + h * 128: k * 256 + (h + 1) * 128], 0, 0)
                    vsb = sbA.tile([128, BT], f32, tag="vsb", name=f"vsb{h}")
                    nc.vector.tensor_scalar_add(vsb[:], acc[:], bv_t[h][:])
                    nc.sync.dma_start(o_vn[h][:], vsb[:])
                    for b in range(B):
                        vtp = psC.tile([128, 128], f32, tag="cp", name=f"vtp{h}_{b}")
                        nc.tensor.transpose(vtp[:], vsb[:, b * 128:(b + 1) * 128], identf[:])
                        nc.scalar.copy(vbf[b][:, h * 129:h * 129 + 128], vtp[:])
                        nc.vector.memset(vbf[b][:, h * 129 + 128:h * 129 + 129], 1.0)

                qk_pass(0)
                v_pass(0)
                qk_pass(1)
                v_pass(1)

                # ---------------- attention (bf16) ----------------
                for h in range(HPC):
                    for b in range(B):
                        kt_t = ktpool.tile([128, TP], bf16, tag="kt", name=f"kt{h}_{b}")
                        nc.sync.dma_start(kt_t[:], p_kt[b, h])
                        at = apool.tile([128, NCH * 128], bf16, tag="at", name=f"at{h}_{b}")

                        # S^T in groups of 4 chunks per psum bank
                        for g in range(0, NCH, 4):
                            grp = [c for c in range(g, min(g + 4, NCH))
                                   if classes[c] != "skip"]
                            if not grp:
                                continue
                            sp = psST.tile([128, 512], f32, tag="sp", name=f"sp{h}_{b}_{g}")
                            for c in grp:
                                j = c - g
                                if c < NCH - 1:
                                    lhsT = kt_t[:, c * 128:(c + 1) * 128]
                                else:
                                    lhsT = kTn[h][:, b * 128:(b + 1) * 128]
                                nc.tensor.matmul(
                                    sp[:, j * 128:(j + 1) * 128], lhsT,
                                    qT[h][:, b * 128:(b + 1) * 128],
                                    start=True, stop=True)
                            j0, j1 = grp[0] - g, grp[-1] - g + 1
                            nc.scalar.activation(
                                at[:, (g + j0) * 128:(g + j1) * 128],
                                sp[:, j0 * 128:j1 * 128],
                                mybir.ActivationFunctionType.Exp,
                                scale=SCALE)
                            for c in grp:
                                if classes[c] == "part":
                                    nc.vector.tensor_mul(
                                        at[:, c * 128:(c + 1) * 128],
                                        at[:, c * 128:(c + 1) * 128],
                                        masks[c][:])

                        vp_t = vppool.tile([128, 16 * 129], bf16, tag="vp", name=f"vp{h}_{b}")
                        nc.sync.dma_start(vp_t[:], p_vp[b, h])
                        op = psO.tile([128, 129], f32, tag="op", name=f"op{h}_{b}")
                        for i, c in enumerate(live_chunks):
                            if c < NCH - 1:
                                rhs = vp_t[:, c * 129:(c + 1) * 129]
                            else:
                                rhs = vbf[b][:, h * 129:(h + 1) * 129]
                            nc.tensor.matmul(
                                op[:], at[:, c * 128:(c + 1) * 128], rhs,
                                start=(i == 0), stop=(i == len(live_chunks) - 1))

                        rec = outsb.tile([128, 1], f32, tag="rec", name=f"rec{h}_{b}")
                        nc.vector.reciprocal(rec[:], op[:, 128:129])
                        osb = outsb.tile([128, 128], bf16, tag="osb", name=f"osb{h}_{b}")
                        nc.vector.tensor_scalar_mul(osb[:], op[:, 0:128], rec[:])
                        otp = psST.tile([128, 128], bf16, tag="sp", name=f"otp{h}_{b}")
                        nc.tensor.transpose(otp[:], osb[:], ident[:])
                        nc.scalar.copy(oT[h][:, b * 128:(b + 1) * 128], otp[:])

                # ---------------- output projection (bf16) ----------------
                for b in range(B):
                    for half in range(2):
                        outt = outsb.tile([128, C // 2], f32, tag="outt",
                                          name=f"outt{b}_{half}", bufs=2)
                        for nn2 in range(2):
                            nn = half * 2 + nn2
                            cp = psC.tile([128, 512], f32, tag="cp", name=f"cp{b}_{nn}")
                            for h in range(HPC):
                                nc.tensor.matmul(
                                    cp[:], oT[h][:, b * 128:(b + 1) * 128],
                                    wo_t[h][:, nn * 512:(nn + 1) * 512],
                                    start=(h == 0), stop=(h == HPC - 1))
                            nc.vector.tensor_copy(outt[:, nn2 * 512:(nn2 + 1) * 512], cp[:])
                        nc.sync.dma_start(
                            o_out[b][:, half * (C // 2):(half + 1) * (C // 2)], outt[:])

    nc.compile()
    return nc


def _prep_inputs(x, K_past, V_past, Wq, bq, Wk, bk, Wv, bv, Wo, bo, off):
    """Build the 8 per-core input maps (host-side packing)."""
    proj_dt = BF16 if PROJ_BF16 else np.float32
    x_flat = np.ascontiguousarray(x.reshape(BT, C)).astype(np.float32)
    xt = np.ascontiguousarray(
        x_flat.T.reshape(KC, 128, BT).transpose(1, 0, 2)).reshape(128, KC * BT).astype(proj_dt)

    ident = np.eye(128, dtype=BF16)
    classes = _chunk_classes(off)
    mask_arrs = {c: _mask_tile(off, c) for c in range(NCH) if classes[c] == "part"}

    in_maps = []
    for core in range(NCORES):
        g0 = core * HPC
        im = {"xt": xt, "ident": ident, "identf": np.eye(128, dtype=np.float32)}
        for c, m in mask_arrs.items():
            im[f"mask{c}"] = m
        for h in range(HPC):
            g = g0 + h
            wq_h = Wq[:, g * D:(g + 1) * D]
            im[f"wq{h}"] = np.ascontiguousarray(
                wq_h.reshape(KC, 128, D).transpose(1, 0, 2)).reshape(128, KC * D).astype(proj_dt)
            wk_h = Wk[:, g * D:(g + 1) * D]
            im[f"wk{h}"] = np.ascontiguousarray(
                wk_h.reshape(KC, 128, D).transpose(1, 0, 2)).reshape(128, KC * D).astype(proj_dt)
            im[f"wo{h}"] = np.ascontiguousarray(Wo[g * D:(g + 1) * D, :]).astype(BF16)
            im[f"bq{h}"] = np.ascontiguousarray(bq[g * D:(g + 1) * D]).reshape(128, 1).astype(np.float32)
            im[f"bk{h}"] = np.ascontiguousarray(bk[g * D:(g + 1) * D]).reshape(128, 1).astype(np.float32)
            im[f"bv{h}"] = np.ascontiguousarray(bv[g * D:(g + 1) * D]).reshape(128, 1).astype(np.float32)
        wv_c = Wv[:, g0 * D:(g0 + HPC) * D]  # (2048, 256)
        im["wv"] = np.ascontiguousarray(
            wv_c.reshape(KC, 128, 256).transpose(1, 0, 2)).reshape(128, KC * 256).astype(proj_dt)
        kp_c = K_past[:, g0:g0 + HPC]        # (B, 2, TP, D)
        im["kt"] = np.ascontiguousarray(kp_c.transpose(0, 1, 3, 2)).astype(BF16)
        vp_c = V_past[:, g0:g0 + HPC]
        vp_l = vp_c.reshape(B, HPC, 16, 128, D).transpose(0, 1, 3, 2, 4)  # (B,2,128,16,D)
        vp = np.empty((B, HPC, 128, 16, 129), dtype=BF16)
        vp[:, :, :, :, :D] = vp_l.astype(BF16)
        vp[:, :, :, :, D] = np.array(1.0, BF16)
        im["vp"] = vp.reshape(B, HPC, 128, 16 * 129)
        in_maps.append(im)
    return in_maps


def kernel(x, K_past, V_past, Wq, bq, Wk, bk, Wv, bv, Wo, bo, position_offset):
    global LAST_EXEC_NS, LAST_RESULTS
    off = int(position_offset)
    x = np.asarray(x, dtype=np.float32)
    K_past = np.asarray(K_past, dtype=np.float32)
    V_past = np.asarray(V_past, dtype=np.float32)
    Wq, Wk, Wv, Wo = (np.asarray(a, dtype=np.float32) for a in (Wq, Wk, Wv, Wo))
    bq, bk, bv, bo = (np.asarray(a, dtype=np.float32) for a in (bq, bk, bv, bo))

    cache_key = (off, PROJ_BF16)
    if cache_key not in _PROGRAM_CACHE:
        _PROGRAM_CACHE[cache_key] = _build_program(off)
    nc = _PROGRAM_CACHE[cache_key]

    in_maps = _prep_inputs(x, K_past, V_past, Wq, bq, Wk, bk, Wv, bv, Wo, bo, off)
    res = run_bass_kernel_spmd(nc, in_maps, list(range(NCORES)), trace=TRACE)
    LAST_EXEC_NS = res.exec_time_ns
    LAST_RESULTS = res

    # ---- host assembly ----
    out = np.zeros((BT, C), np.float32)
    for core in range(NCORES):
        out += res.results[core]["out_partial"].reshape(BT, C)
    out += bo[None, :]
    out = out.reshape(B, T, C)

    K_full = np.empty((B, H, TT, D), np.float32)
    V_full = np.empty((B, H, TT, D), np.float32)
    K_full[:, :, :TP] = K_past
    V_full[:, :, :TP] = V_past
    for core in range(NCORES):
        for h in range(HPC):
            g = core * HPC + h
            kn = res.results[core][f"knew{h}"]          # (128 d, BT)
            K_full[:, g, TP:] = kn.reshape(D, B, T).transpose(1, 2, 0)
            vn = res.results[core][f"vnewT{h}"]         # (128 d, BT)
            V_full[:, g, TP:] = vn.reshape(D, B, T).transpose(1, 2, 0)
    return out, K_full, V_full


# revision 26
# speedup vs baseline: 1.1822x; 1.0353x over previous
"""Trainium2 Bass kernel for multi-head attention with KV cache.

Problem: B=8, T_new=128, C=2048, H=16, D=128, T_past=2048.
Returns (out, K, V) like the reference:
    Q/K/V = x @ W* + b*  (split heads)
    K/V caches = concat(past, new)
    out = softmax(Q K^T / sqrt(D) + causal) V  -> @ Wo + bo

Sharding: tensor-parallel over heads, 2 heads per core x 8 cores.
Each core computes Q/K/V projections for its 2 heads (fp32r), runs
attention over all 8 batches (bf16 internals, fp32 accumulate), and
produces a partial output projection (bf16); host sums the 8 partials.

Projections all run d-major (out = W_chunk.T @ x^T chunk) as six
accumulation passes (Qh0 Kh0 Qh1 Kh1 Vd0 Vd1) of shape (128, B*T).
V is then PE-transposed per 128-block into the natural t-major layout
used by the attention O matmul (with a ones column appended so the
softmax denominators fall out of the same matmul).

Attention dataflow (per batch b, head h):
  S^T[t,q]  = sum_d KT[d,t] * QT[d,q]        (17 chunks of 128 t-rows)
  A^T       = exp(S^T * 1/sqrt(D) + mask)    (ACT engine, PSUM->SBUF bf16)
  O[q,0:128], O[q,128] = sum_t A^T[t,q] * [V[t,:] | 1]
  O_norm    = O[:, :128] * recip(O[:,128])
  O^T       = PE-transpose(O_norm)           -> out-projection lhsT
"""

import os
import sys

sys.path.insert(0, "/opt/trn_rl_repo")

import numpy as np
import ml_dtypes

import concourse.bass as bass
import concourse.tile as tile
from concourse import mybir, bacc
from concourse.bass_utils import run_bass_kernel_spmd

BF16 = ml_dtypes.bfloat16

B, T, C = 8, 128, 2048
H, TP, D = 16, 2048, 128
NCORES = 8
HPC = H // NCORES          # heads per core = 2
TT = TP + T                # total keys = 2176
NCH = TT // 128            # 17 chunks of keys
BT = B * T                 # 1024
KC = C // 128              # 16 contraction chunks
KVW = TP + 16 * 129        # packed K^T | V-chunks width = 4112
SCALE = 1.0 / float(np.sqrt(D))
NEG = -1.0e9

# module-level knobs for test.py
TRACE = False
LAST_EXEC_NS = None
LAST_RESULTS = None
PROJ_BF16 = True   # bf16 projections (faster); False = fp32r (K/V ~1.3e-4)

_PROGRAM_CACHE = {}


def _chunk_classes(off):
    """Classify each key chunk: 'full' (all visible), 'part', 'skip'."""
    classes = []
    for c in range(NCH):
        if c < NCH - 1:
            kg0, kg1 = c * 128, c * 128 + 127
        else:
            kg0, kg1 = off, off + 127
        if kg1 <= off:                      # visible for every query
            classes.append("full")
        elif kg0 > off + 127:               # hidden for every query
            classes.append("skip")
        else:
            classes.append("part")
    return classes


def _mask_tile(off, c):
    """Additive fp32 mask (128 t x 128 q) for a 'part' chunk."""
    tt = np.arange(128)
    kg = (c * 128 + tt) if c < NCH - 1 else (off + tt)
    q = off + np.arange(128)
    vis = kg[:, None] <= q[None, :]
    return np.where(vis, np.float32(1.0), np.float32(0.0)).astype(BF16)


def _build_program(off):
    classes = _chunk_classes(off)
    part_chunks = [c for c in range(NCH) if classes[c] == "part"]
    live_chunks = [c for c in range(NCH) if classes[c] != "skip"]

    nc = bacc.Bacc("TRN2", target_bir_lowering=False, debug=False,
                   num_devices=NCORES)
    f32 = mybir.dt.float32
    f32r = mybir.dt.bfloat16 if PROJ_BF16 else mybir.dt.float32r
    bf16 = mybir.dt.bfloat16

    # ---- DRAM parameters (per-core data, host-packed layouts) ----
    p_xt = nc.declare_dram_parameter("xt", [128, KC * BT], f32r, isOutput=False)
    p_wq = [nc.declare_dram_parameter(f"wq{h}", [128, KC * 128], f32r, isOutput=False) for h in range(HPC)]
    p_wk = [nc.declare_dram_parameter(f"wk{h}", [128, KC * 128], f32r, isOutput=False) for h in range(HPC)]
    p_wv = nc.declare_dram_parameter("wv", [128, KC * 256], f32r, isOutput=False)
    p_wo = [nc.declare_dram_parameter(f"wo{h}", [128, C], bf16, isOutput=False) for h in range(HPC)]
    p_bq = [nc.declare_dram_parameter(f"bq{h}", [128, 1], f32, isOutput=False) for h in range(HPC)]
    p_bk = [nc.declare_dram_parameter(f"bk{h}", [128, 1], f32, isOutput=False) for h in range(HPC)]
    p_bv = [nc.declare_dram_parameter(f"bv{h}", [128, 1], f32, isOutput=False) for h in range(HPC)]
    # per (b,h): K^T (128 x 2048) and V packed with ones col (128 x 2064)
    p_kt = nc.declare_dram_parameter("kt", [B, HPC, 128, TP], bf16, isOutput=False)
    p_vp = nc.declare_dram_parameter("vp", [B, HPC, 128, 16 * 129], bf16, isOutput=False)
    p_id = nc.declare_dram_parameter("ident", [128, 128], bf16, isOutput=False)
    p_idf = nc.declare_dram_parameter("identf", [128, 128], f32, isOutput=False)
    p_mask = {c: nc.declare_dram_parameter(f"mask{c}", [128, 128], bf16, isOutput=False) for c in part_chunks}

    o_out = nc.declare_dram_parameter("out_partial", [B, 128, C], f32, isOutput=True)
    o_kn = [nc.declare_dram_parameter(f"knew{h}", [128, BT], f32, isOutput=True) for h in range(HPC)]
    o_vn = [nc.declare_dram_parameter(f"vnewT{h}", [128, BT], f32, isOutput=True) for h in range(HPC)]

    with tile.TileContext(nc) as tc:
        with tc.tile_pool(name="sbA", bufs=1) as sbA, \
             tc.tile_pool(name="const", bufs=1) as constp, \
             tc.tile_pool(name="pers", bufs=1) as pers, \
             tc.tile_pool(name="ktstream", bufs=8) as ktpool, \
             tc.tile_pool(name="vpstream", bufs=8) as vppool, \
             tc.tile_pool(name="astream", bufs=4) as apool, \
             tc.tile_pool(name="outsb", bufs=2) as outsb:

            # --- phase A inputs first in trace order (weights, then x^T) ---
            wq_t, wk_t = [], []
            for h in range(HPC):
                w = sbA.tile([128, KC * 128], f32r, tag=f"wq{h}", name=f"wqt{h}")
                nc.sync.dma_start(w[:], p_wq[h][:])
                wq_t.append(w)
                w = sbA.tile([128, KC * 128], f32r, tag=f"wk{h}", name=f"wkt{h}")
                nc.sync.dma_start(w[:], p_wk[h][:])
                wk_t.append(w)
            xt = [sbA.tile([128, BT], f32r, tag=f"xt{k}", name=f"xt{k}") for k in range(KC)]
            for k in range(KC):
                nc.sync.dma_start(xt[k][:], p_xt[:, k * BT:(k + 1) * BT])
            wv_t = sbA.tile([128, KC * 256], f32r, tag="wv")
            nc.sync.dma_start(wv_t[:], p_wv[:])

            # --- small constants ---
            ident = constp.tile([128, 128], bf16, tag="ident")
            nc.sync.dma_start(ident[:], p_id[:])
            identf = constp.tile([128, 128], f32, tag="identf")
            nc.sync.dma_start(identf[:], p_idf[:])
            masks = {}
            for c in part_chunks:
                mt = constp.tile([128, 128], bf16, tag=f"mask{c}", name=f"mask{c}")
                nc.sync.dma_start(mt[:], p_mask[c][:])
                masks[c] = mt
            bq_t, bk_t, bv_t = [], [], []
            for h in range(HPC):
                t1 = constp.tile([128, 1], f32, tag=f"bq{h}", name=f"bqt{h}")
                nc.sync.dma_start(t1[:], p_bq[h][:])
                bq_t.append(t1)
                t2 = constp.tile([128, 1], f32, tag=f"bk{h}", name=f"bkt{h}")
                nc.sync.dma_start(t2[:], p_bk[h][:])
                bk_t.append(t2)
                t3 = constp.tile([128, 1], f32, tag=f"bv{h}", name=f"bvt{h}")
                nc.sync.dma_start(t3[:], p_bv[h][:])
                bv_t.append(t3)
            wo_t = []
            for h in range(HPC):
                w = constp.tile([128, C], bf16, tag=f"wo{h}", name=f"wot{h}")
                nc.sync.dma_start(w[:], p_wo[h][:])
                wo_t.append(w)

            qT = [pers.tile([128, BT], bf16, tag=f"qT{h}", name=f"qT{h}") for h in range(HPC)]
            kTn = [pers.tile([128, BT], bf16, tag=f"kTn{h}", name=f"kTn{h}") for h in range(HPC)]
            vbf = [pers.tile([128, 258], bf16, tag=f"vbf{b}", name=f"vbf{b}") for b in range(B)]
            oT = [pers.tile([128, BT], bf16, tag=f"oT{h}", name=f"oT{h}") for h in range(HPC)]

            with tc.tile_pool(name="psA", bufs=2, space="PSUM") as psA, \
                 tc.tile_pool(name="psST", bufs=2, space="PSUM") as psST, \
                 tc.tile_pool(name="psO", bufs=2, space="PSUM") as psO, \
                 tc.tile_pool(name="psC", bufs=2, space="PSUM") as psC:

                def half_pass(name, w_ap, dst_ops):
                    """One (128, BT) projection pass over the 16 c-chunks,
                    as two (128,512) half-accumulations. dst_ops(acc, nn)
                    drains the finished half."""
                    for nn in range(2):
                        acc = psA.tile([128, 512], f32, tag="acc", name=f"acc_{name}_{nn}")
                        for k in range(KC):
                            nc.tensor.matmul(
                                acc[:],
                                w_ap(k),
                                xt[k][:, nn * 512:(nn + 1) * 512],
                                start=(k == 0), stop=(k == KC - 1))
                        dst_ops(acc, nn)

                ksb = [sbA.tile([128, BT], f32, tag=f"ksb{h}", name=f"ksb{h}") for h in range(HPC)]
                vsb = [sbA.tile([128, BT], f32, tag=f"vsb{h}", name=f"vsb{h}") for h in range(HPC)]

                def qk_pass(h):
                    def drain_q(acc, nn):
                        nc.vector.tensor_scalar_add(
                            qT[h][:, nn * 512:(nn + 1) * 512], acc[:], bq_t[h][:])
                    half_pass(f"q{h}", lambda k, h=h: wq_t[h][:, k * 128:(k + 1) * 128], drain_q)

                    def drain_k(acc, nn):
                        nc.vector.tensor_scalar_add(
                            kTn[h][:, nn * 512:(nn + 1) * 512], acc[:], bk_t[h][:])
                        nc.vector.tensor_scalar_add(
                            ksb[h][:, nn * 512:(nn + 1) * 512], acc[:], bk_t[h][:])
                    half_pass(f"k{h}", lambda k, h=h: wk_t[h][:, k * 128:(k + 1) * 128], drain_k)
                    nc.sync.dma_start(o_kn[h][:], ksb[h][:])

                def v_pass(h):
                    def drain_v(acc, nn):
                        nc.vector.tensor_scalar_add(
                            vsb[h][:, nn * 512:(nn + 1) * 512], acc[:], bv_t[h][:])
                    half_pass(f"v{h}", lambda k, h=h: wv_t[:, k * 256 + h * 128: k * 256 + (h + 1) * 128], drain_v)
                    nc.sync.dma_start(o_vn[h][:], vsb[h][:])
                    for b in range(B):
                        vtp = psC.tile([128, 128], f32, tag="cp", name=f"vtp{h}_{b}")
                        nc.tensor.transpose(vtp[:], vsb[h][:, b * 128:(b + 1) * 128], identf[:])
                        nc.scalar.copy(vbf[b][:, h * 129:h * 129 + 128], vtp[:])
                        nc.vector.memset(vbf[b][:, h * 129 + 128:h * 129 + 129], 1.0)

                qk_pass(0)
                qk_pass(1)
                v_pass(0)
                v_pass(1)

                # ---- attention: software-pipelined over 16 (b,h) units ----
                units = [(b, h) for b in range(B) for h in range(HPC)]
                NU = len(units)
                at_t = [None] * NU
                op_t = [None] * NU
                osb_t = [None] * NU
                vp_tl = [None] * NU

                def stage_S(i):
                    b, h = units[i]
                    kt_t = ktpool.tile([128, TP], bf16, tag="kt", name=f"kt{i}")
                    nc.sync.dma_start(kt_t[:], p_kt[b, h])
                    vp_tl[i] = vppool.tile([128, 16 * 129], bf16, tag="vp", name=f"vp{i}")
                    nc.sync.dma_start(vp_tl[i][:], p_vp[b, h])
                    at = apool.tile([128, NCH * 128], bf16, tag="at", name=f"at{i}")
                    at_t[i] = at
                    for g in range(0, NCH, 4):
                        grp = [c for c in range(g, min(g + 4, NCH))
                               if classes[c] != "skip"]
                        if not grp:
                            continue
                        sp = psST.tile([128, 512], f32, tag="sp", name=f"sp{i}_{g}")
                        for c in grp:
                            j = c - g
                            if c < NCH - 1:
                                lhsT = kt_t[:, c * 128:(c + 1) * 128]
                            else:
                                lhsT = kTn[h][:, b * 128:(b + 1) * 128]
                            nc.tensor.matmul(
                                sp[:, j * 128:(j + 1) * 128], lhsT,
                                qT[h][:, b * 128:(b + 1) * 128],
                                start=True, stop=True)
                        j0, j1 = grp[0] - g, grp[-1] - g + 1
                        nc.scalar.activation(
                            at[:, (g + j0) * 128:(g + j1) * 128],
                            sp[:, j0 * 128:j1 * 128],
                            mybir.ActivationFunctionType.Exp,
                            scale=SCALE)
                        for c in grp:
                            if classes[c] == "part":
                                nc.vector.tensor_mul(
                                    at[:, c * 128:(c + 1) * 128],
                                    at[:, c * 128:(c + 1) * 128],
                                    masks[c][:])

                def stage_O(i):
                    b, h = units[i]
                    at = at_t[i]
                    op = psO.tile([128, 129], f32, tag="op", name=f"op{i}")
                    op_t[i] = op
                    for j, c in enumerate(live_chunks):
                        if c < NCH - 1:
                            rhs = vp_tl[i][:, c * 129:(c + 1) * 129]
                        else:
                            rhs = vbf[b][:, h * 129:(h + 1) * 129]
                        nc.tensor.matmul(
                            op[:], at[:, c * 128:(c + 1) * 128], rhs,
                            start=(j == 0), stop=(j == len(live_chunks) - 1))
                    rec = outsb.tile([128, 1], f32, tag="rec", name=f"rec{i}")
                    nc.vector.reciprocal(rec[:], op[:, 128:129])
                    osb = outsb.tile([128, 128], bf16, tag="osb", name=f"osb{i}", bufs=3)
                    osb_t[i] = osb
                    nc.vector.tensor_scalar_mul(osb[:], op[:, 0:128], rec[:])

                def stage_T(i):
                    b, h = units[i]
                    otp = psST.tile([128, 128], bf16, tag="sp", name=f"otp{i}")
                    nc.tensor.transpose(otp[:], osb_t[i][:], ident[:])
                    nc.scalar.copy(oT[h][:, b * 128:(b + 1) * 128], otp[:])

                def stage_C(b):
                    for half in range(2):
                        outt = outsb.tile([128, C // 2], f32, tag="outt",
                                          name=f"outt{b}_{half}", bufs=2)
                        for nn2 in range(2):
                            nn = half * 2 + nn2
                            cp = psC.tile([128, 512], f32, tag="cp", name=f"cp{b}_{nn}")
                            for h in range(HPC):
                                nc.tensor.matmul(
                                    cp[:], oT[h][:, b * 128:(b + 1) * 128],
                                    wo_t[h][:, nn * 512:(nn + 1) * 512],
                                    start=(h == 0), stop=(h == HPC - 1))
                            nc.vector.tensor_copy(outt[:, nn2 * 512:(nn2 + 1) * 512], cp[:])
                        nc.sync.dma_start(
                            o_out[b][:, half * (C // 2):(half + 1) * (C // 2)], outt[:])

                # stagger: S(i+1) ahead of O(i); T lags O by one more unit;
                # C(b) as soon as both its heads' transposes are done.
                for i in range(NU + 2):
                    if i < NU:
                        stage_S(i)
                    if 1 <= i <= NU:
                        stage_O(i - 1)
                    if 2 <= i <= NU + 1:
                        stage_T(i - 2)
                        b, h = units[i - 2]
                        if h == HPC - 1:
                            stage_C(b)

    nc.compile()
    return nc


def _prep_inputs(x, K_past, V_past, Wq, bq, Wk, bk, Wv, bv, Wo, bo, off):
    """Build the 8 per-core input maps (host-side packing)."""
    proj_dt = BF16 if PROJ_BF16 else np.float32
    x_flat = np.ascontiguousarray(x.reshape(BT, C)).astype(np.float32)
    xt = np.ascontiguousarray(
        x_flat.T.reshape(KC, 128, BT).transpose(1, 0, 2)).reshape(128, KC * BT).astype(proj_dt)

    ident = np.eye(128, dtype=BF16)
    classes = _chunk_classes(off)
    mask_arrs = {c: _mask_tile(off, c) for c in range(NCH) if classes[c] == "part"}

    in_maps = []
    for core in range(NCORES):
        g0 = core * HPC
        im = {"xt": xt, "ident": ident, "identf": np.eye(128, dtype=np.float32)}
        for c, m in mask_arrs.items():
            im[f"mask{c}"] = m
        for h in range(HPC):
            g = g0 + h
            wq_h = Wq[:, g * D:(g + 1) * D]
            im[f"wq{h}"] = np.ascontiguousarray(
                wq_h.reshape(KC, 128, D).transpose(1, 0, 2)).reshape(128, KC * D).astype(proj_dt)
            wk_h = Wk[:, g * D:(g + 1) * D]
            im[f"wk{h}"] = np.ascontiguousarray(
                wk_h.reshape(KC, 128, D).transpose(1, 0, 2)).reshape(128, KC * D).astype(proj_dt)
            im[f"wo{h}"] = np.ascontiguousarray(Wo[g * D:(g + 1) * D, :]).astype(BF16)
            im[f"bq{h}"] = np.ascontiguousarray(bq[g * D:(g + 1) * D]).reshape(128, 1).astype(np.float32)
            im[f"bk{h}"] = np.ascontiguousarray(bk[g * D:(g + 1) * D]).reshape(128, 1).astype(np.float32)
            im[f"bv{h}"] = np.ascontiguousarray(bv[g * D:(g + 1) * D]).reshape(128, 1).astype(np.float32)
        wv_c = Wv[:, g0 * D:(g0 + HPC) * D]  # (2048, 256)
        im["wv"] = np.ascontiguousarray(
            wv_c.reshape(KC, 128, 256).transpose(1, 0, 2)).reshape(128, KC * 256).astype(proj_dt)
        kp_c = K_past[:, g0:g0 + HPC]        # (B, 2, TP, D)
        im["kt"] = np.ascontiguousarray(kp_c.transpose(0, 1, 3, 2)).astype(BF16)
        vp_c = V_past[:, g0:g0 + HPC]
        vp_l = vp_c.reshape(B, HPC, 16, 128, D).transpose(0, 1, 3, 2, 4)  # (B,2,128,16,D)
        vp = np.empty((B, HPC, 128, 16, 129), dtype=BF16)
        vp[:, :, :, :, :D] = vp_l.astype(BF16)
        vp[:, :, :, :, D] = np.array(1.0, BF16)
        im["vp"] = vp.reshape(B, HPC, 128, 16 * 129)
        in_maps.append(im)
    return in_maps


def kernel(x, K_past, V_past, Wq, bq, Wk, bk, Wv, bv, Wo, bo, position_offset):
    global LAST_EXEC_NS, LAST_RESULTS
    off = int(position_offset)
    x = np.asarray(x, dtype=np.float32)
    K_past = np.asarray(K_past, dtype=np.float32)
    V_past = np.asarray(V_past, dtype=np.float32)
    Wq, Wk, Wv, Wo = (np.asarray(a, dtype=np.float32) for a in (Wq, Wk, Wv, Wo))
    bq, bk, bv, bo = (np.asarray(a, dtype=np.float32) for a in (bq, bk, bv, bo))

    cache_key = (off, PROJ_BF16)
    if cache_key not in _PROGRAM_CACHE:
        _PROGRAM_CACHE[cache_key] = _build_program(off)
    nc = _PROGRAM_CACHE[cache_key]

    in_maps = _prep_inputs(x, K_past, V_past, Wq, bq, Wk, bk, Wv, bv, Wo, bo, off)
    res = run_bass_kernel_spmd(nc, in_maps, list(range(NCORES)), trace=TRACE)
    LAST_EXEC_NS = res.exec_time_ns
    LAST_RESULTS = res

    # ---- host assembly ----
    out = np.zeros((BT, C), np.float32)
    for core in range(NCORES):
        out += res.results[core]["out_partial"].reshape(BT, C)
    out += bo[None, :]
    out = out.reshape(B, T, C)

    K_full = np.empty((B, H, TT, D), np.float32)
    V_full = np.empty((B, H, TT, D), np.float32)
    K_full[:, :, :TP] = K_past
    V_full[:, :, :TP] = V_past
    for core in range(NCORES):
        for h in range(HPC):
            g = core * HPC + h
            kn = res.results[core][f"knew{h}"]          # (128 d, BT)
            K_full[:, g, TP:] = kn.reshape(D, B, T).transpose(1, 2, 0)
            vn = res.results[core][f"vnewT{h}"]         # (128 d, BT)
            V_full[:, g, TP:] = vn.reshape(D, B, T).transpose(1, 2, 0)
    return out, K_full, V_full


# revision 27
# speedup vs baseline: 1.4052x; 1.1886x over previous
"""Trainium2 Bass kernel for multi-head attention with KV cache.

Problem: B=8, T_new=128, C=2048, H=16, D=128, T_past=2048.
Returns (out, K, V) like the reference:
    Q/K/V = x @ W* + b*  (split heads)
    K/V caches = concat(past, new)
    out = softmax(Q K^T / sqrt(D) + causal) V  -> @ Wo + bo

Sharding: tensor-parallel over heads, 2 heads per core x 8 cores.
Each core computes Q/K/V projections for its 2 heads (fp32r), runs
attention over all 8 batches (bf16 internals, fp32 accumulate), and
produces a partial output projection (bf16); host sums the 8 partials.

Projections all run d-major (out = W_chunk.T @ x^T chunk) as six
accumulation passes (Qh0 Kh0 Qh1 Kh1 Vd0 Vd1) of shape (128, B*T).
V is then PE-transposed per 128-block into the natural t-major layout
used by the attention O matmul (with a ones column appended so the
softmax denominators fall out of the same matmul).

Attention dataflow (per batch b, head h):
  S^T[t,q]  = sum_d KT[d,t] * QT[d,q]        (17 chunks of 128 t-rows)
  A^T       = exp(S^T * 1/sqrt(D) + mask)    (ACT engine, PSUM->SBUF bf16)
  O[q,0:128], O[q,128] = sum_t A^T[t,q] * [V[t,:] | 1]
  O_norm    = O[:, :128] * recip(O[:,128])
  O^T       = PE-transpose(O_norm)           -> out-projection lhsT
"""

import os
import sys

sys.path.insert(0, "/opt/trn_rl_repo")

import numpy as np
import ml_dtypes

import concourse.bass as bass
import concourse.tile as tile
from concourse import mybir, bacc
from concourse.bass_utils import run_bass_kernel_spmd

BF16 = ml_dtypes.bfloat16

B, T, C = 8, 128, 2048
H, TP, D = 16, 2048, 128
NCORES = 8
HPC = H // NCORES          # heads per core = 2
TT = TP + T                # total keys = 2176
NCH = TT // 128            # 17 chunks of keys
BT = B * T                 # 1024
KC = C // 128              # 16 contraction chunks
KVW = TP + 16 * 129        # packed K^T | V-chunks width = 4112
SCALE = 1.0 / float(np.sqrt(D))
NEG = -1.0e9

# module-level knobs for test.py
TRACE = False
LAST_EXEC_NS = None
LAST_RESULTS = None
PROJ_BF16 = True   # bf16 projections (faster); False = fp32r (K/V ~1.3e-4)

_PROGRAM_CACHE = {}


def _chunk_classes(off):
    """Classify each key chunk: 'full' (all visible), 'part', 'skip'."""
    classes = []
    for c in range(NCH):
        if c < NCH - 1:
            kg0, kg1 = c * 128, c * 128 + 127
        else:
            kg0, kg1 = off, off + 127
        if kg1 <= off:                      # visible for every query
            classes.append("full")
        elif kg0 > off + 127:               # hidden for every query
            classes.append("skip")
        else:
            classes.append("part")
    return classes


def _mask_tile(off, c):
    """Additive fp32 mask (128 t x 128 q) for a 'part' chunk."""
    tt = np.arange(128)
    kg = (c * 128 + tt) if c < NCH - 1 else (off + tt)
    q = off + np.arange(128)
    vis = kg[:, None] <= q[None, :]
    return np.where(vis, np.float32(1.0), np.float32(0.0)).astype(BF16)


def _build_program(off):
    classes = _chunk_classes(off)
    part_chunks = [c for c in range(NCH) if classes[c] == "part"]
    live_chunks = [c for c in range(NCH) if classes[c] != "skip"]

    nc = bacc.Bacc("TRN2", target_bir_lowering=False, debug=False,
                   num_devices=NCORES)
    f32 = mybir.dt.float32
    f32r = mybir.dt.bfloat16 if PROJ_BF16 else mybir.dt.float32r
    bf16 = mybir.dt.bfloat16

    # ---- DRAM parameters (per-core data, host-packed layouts) ----
    p_xt = nc.declare_dram_parameter("xt", [128, KC * BT], f32r, isOutput=False)
    p_wq = [nc.declare_dram_parameter(f"wq{h}", [128, KC * 128], f32r, isOutput=False) for h in range(HPC)]
    p_wk = [nc.declare_dram_parameter(f"wk{h}", [128, KC * 128], f32r, isOutput=False) for h in range(HPC)]
    p_wv = nc.declare_dram_parameter("wv", [128, KC * 256], f32r, isOutput=False)
    p_wo = [nc.declare_dram_parameter(f"wo{h}", [128, C], bf16, isOutput=False) for h in range(HPC)]
    p_bq = [nc.declare_dram_parameter(f"bq{h}", [128, 1], f32, isOutput=False) for h in range(HPC)]
    p_bk = [nc.declare_dram_parameter(f"bk{h}", [128, 1], f32, isOutput=False) for h in range(HPC)]
    p_bv = [nc.declare_dram_parameter(f"bv{h}", [128, 1], f32, isOutput=False) for h in range(HPC)]
    # per (b,h): K^T (128 x 2048) and V packed with ones col (128 x 2064)
    p_kt = nc.declare_dram_parameter("kt", [B, HPC, 128, TP], bf16, isOutput=False)
    p_vp = nc.declare_dram_parameter("vp", [B, HPC, 128, 16 * 129], bf16, isOutput=False)
    p_id = nc.declare_dram_parameter("ident", [128, 128], bf16, isOutput=False)
    p_idf = nc.declare_dram_parameter("identf", [128, 128], f32, isOutput=False)
    p_mask = {c: nc.declare_dram_parameter(f"mask{c}", [128, 128], bf16, isOutput=False) for c in part_chunks}

    o_out = nc.declare_dram_parameter("out_partial", [B, 128, C], bf16, isOutput=True)
    o_kn = [nc.declare_dram_parameter(f"knew{h}", [128, BT], f32, isOutput=True) for h in range(HPC)]
    o_vn = [nc.declare_dram_parameter(f"vnewT{h}", [128, BT], f32, isOutput=True) for h in range(HPC)]

    with tile.TileContext(nc) as tc:
        with tc.tile_pool(name="sbA", bufs=1) as sbA, \
             tc.tile_pool(name="const", bufs=1) as constp, \
             tc.tile_pool(name="pers", bufs=1) as pers, \
             tc.tile_pool(name="ktstream", bufs=8) as ktpool, \
             tc.tile_pool(name="vpstream", bufs=8) as vppool, \
             tc.tile_pool(name="astream", bufs=5) as apool, \
             tc.tile_pool(name="outsb", bufs=2) as outsb:

            # --- phase A inputs first in trace order (weights, then x^T) ---
            wq_t, wk_t = [], []
            for h in range(HPC):
                w = sbA.tile([128, KC * 128], f32r, tag=f"wq{h}", name=f"wqt{h}")
                nc.sync.dma_start(w[:], p_wq[h][:])
                wq_t.append(w)
                w = sbA.tile([128, KC * 128], f32r, tag=f"wk{h}", name=f"wkt{h}")
                nc.sync.dma_start(w[:], p_wk[h][:])
                wk_t.append(w)
            xt = [sbA.tile([128, BT], f32r, tag=f"xt{k}", name=f"xt{k}") for k in range(KC)]
            for k in range(KC):
                nc.sync.dma_start(xt[k][:], p_xt[:, k * BT:(k + 1) * BT])
            wv_t = sbA.tile([128, KC * 256], f32r, tag="wv")
            nc.sync.dma_start(wv_t[:], p_wv[:])

            # --- small constants ---
            ident = constp.tile([128, 128], bf16, tag="ident")
            nc.sync.dma_start(ident[:], p_id[:])
            identf = constp.tile([128, 128], f32, tag="identf")
            nc.sync.dma_start(identf[:], p_idf[:])
            masks = {}
            for c in part_chunks:
                mt = constp.tile([128, 128], bf16, tag=f"mask{c}", name=f"mask{c}")
                nc.sync.dma_start(mt[:], p_mask[c][:])
                masks[c] = mt
            bq_t, bk_t, bv_t = [], [], []
            for h in range(HPC):
                t1 = constp.tile([128, 1], f32, tag=f"bq{h}", name=f"bqt{h}")
                nc.sync.dma_start(t1[:], p_bq[h][:])
                bq_t.append(t1)
                t2 = constp.tile([128, 1], f32, tag=f"bk{h}", name=f"bkt{h}")
                nc.sync.dma_start(t2[:], p_bk[h][:])
                bk_t.append(t2)
                t3 = constp.tile([128, 1], f32, tag=f"bv{h}", name=f"bvt{h}")
                nc.sync.dma_start(t3[:], p_bv[h][:])
                bv_t.append(t3)
            wo_t = []
            for h in range(HPC):
                w = constp.tile([128, C], bf16, tag=f"wo{h}", name=f"wot{h}")
                nc.sync.dma_start(w[:], p_wo[h][:])
                wo_t.append(w)

            qT = [pers.tile([128, BT], bf16, tag=f"qT{h}", name=f"qT{h}") for h in range(HPC)]
            kTn = [pers.tile([128, BT], bf16, tag=f"kTn{h}", name=f"kTn{h}") for h in range(HPC)]
            vbf = [pers.tile([128, 258], bf16, tag=f"vbf{b}", name=f"vbf{b}") for b in range(B)]
            oT = [pers.tile([128, BT], bf16, tag=f"oT{h}", name=f"oT{h}") for h in range(HPC)]

            with tc.tile_pool(name="psA", bufs=2, space="PSUM") as psA, \
                 tc.tile_pool(name="psST", bufs=2, space="PSUM") as psST, \
                 tc.tile_pool(name="psO", bufs=2, space="PSUM") as psO, \
                 tc.tile_pool(name="psC", bufs=2, space="PSUM") as psC:

                def half_pass(name, w_ap, dst_ops):
                    """One (128, BT) projection pass over the 16 c-chunks,
                    as two (128,512) half-accumulations. dst_ops(acc, nn)
                    drains the finished half."""
                    for nn in range(2):
                        acc = psA.tile([128, 512], f32, tag="acc", name=f"acc_{name}_{nn}")
                        for k in range(KC):
                            nc.tensor.matmul(
                                acc[:],
                                w_ap(k),
                                xt[k][:, nn * 512:(nn + 1) * 512],
                                start=(k == 0), stop=(k == KC - 1))
                        dst_ops(acc, nn)

                ksb = [sbA.tile([128, BT], f32, tag=f"ksb{h}", name=f"ksb{h}") for h in range(HPC)]
                vsb = [sbA.tile([128, BT], f32, tag=f"vsb{h}", name=f"vsb{h}") for h in range(HPC)]

                def qk_pass(h):
                    def drain_q(acc, nn):
                        nc.vector.tensor_scalar_add(
                            qT[h][:, nn * 512:(nn + 1) * 512], acc[:], bq_t[h][:])
                    half_pass(f"q{h}", lambda k, h=h: wq_t[h][:, k * 128:(k + 1) * 128], drain_q)

                    def drain_k(acc, nn):
                        nc.vector.tensor_scalar_add(
                            kTn[h][:, nn * 512:(nn + 1) * 512], acc[:], bk_t[h][:])
                        nc.vector.tensor_scalar_add(
                            ksb[h][:, nn * 512:(nn + 1) * 512], acc[:], bk_t[h][:])
                    half_pass(f"k{h}", lambda k, h=h: wk_t[h][:, k * 128:(k + 1) * 128], drain_k)
                    nc.sync.dma_start(o_kn[h][:], ksb[h][:])

                def v_pass(h):
                    def drain_v(acc, nn):
                        nc.vector.tensor_scalar_add(
                            vsb[h][:, nn * 512:(nn + 1) * 512], acc[:], bv_t[h][:])
                    half_pass(f"v{h}", lambda k, h=h: wv_t[:, k * 256 + h * 128: k * 256 + (h + 1) * 128], drain_v)
                    nc.sync.dma_start(o_vn[h][:], vsb[h][:])
                    for b in range(B):
                        vtp = psC.tile([128, 128], f32, tag="cp", name=f"vtp{h}_{b}")
                        nc.tensor.transpose(vtp[:], vsb[h][:, b * 128:(b + 1) * 128], identf[:])
                        nc.scalar.copy(vbf[b][:, h * 129:h * 129 + 128], vtp[:])
                        nc.vector.memset(vbf[b][:, h * 129 + 128:h * 129 + 129], 1.0)

                # ---- attention: software-pipelined over 16 (b,h) units ----
                units = [(b, h) for b in range(B) for h in range(HPC)]
                NU = len(units)
                at_t = [None] * NU
                op_t = [None] * NU
                osb_t = [None] * NU
                vp_tl = [None] * NU

                def stage_S(i):
                    b, h = units[i]
                    kt_t = ktpool.tile([128, TP], bf16, tag="kt", name=f"kt{i}")
                    nc.sync.dma_start(kt_t[:], p_kt[b, h])
                    vp_tl[i] = vppool.tile([128, 16 * 129], bf16, tag="vp", name=f"vp{i}")
                    nc.sync.dma_start(vp_tl[i][:], p_vp[b, h])
                    at = apool.tile([128, NCH * 128], bf16, tag="at", name=f"at{i}")
                    at_t[i] = at
                    for g in range(0, NCH, 4):
                        grp = [c for c in range(g, min(g + 4, NCH))
                               if classes[c] != "skip"]
                        if not grp:
                            continue
                        sp = psST.tile([128, 512], f32, tag="sp", name=f"sp{i}_{g}")
                        for c in grp:
                            j = c - g
                            if c < NCH - 1:
                                lhsT = kt_t[:, c * 128:(c + 1) * 128]
                            else:
                                lhsT = kTn[h][:, b * 128:(b + 1) * 128]
                            nc.tensor.matmul(
                                sp[:, j * 128:(j + 1) * 128], lhsT,
                                qT[h][:, b * 128:(b + 1) * 128],
                                start=True, stop=True)
                        j0, j1 = grp[0] - g, grp[-1] - g + 1
                        nc.scalar.activation(
                            at[:, (g + j0) * 128:(g + j1) * 128],
                            sp[:, j0 * 128:j1 * 128],
                            mybir.ActivationFunctionType.Exp,
                            scale=SCALE)
                        for c in grp:
                            if classes[c] == "part":
                                nc.vector.tensor_mul(
                                    at[:, c * 128:(c + 1) * 128],
                                    at[:, c * 128:(c + 1) * 128],
                                    masks[c][:])

                def stage_O(i):
                    b, h = units[i]
                    at = at_t[i]
                    op = psO.tile([128, 129], f32, tag="op", name=f"op{i}")
                    op_t[i] = op
                    for j, c in enumerate(live_chunks):
                        if c < NCH - 1:
                            rhs = vp_tl[i][:, c * 129:(c + 1) * 129]
                        else:
                            rhs = vbf[b][:, h * 129:(h + 1) * 129]
                        nc.tensor.matmul(
                            op[:], at[:, c * 128:(c + 1) * 128], rhs,
                            start=(j == 0), stop=(j == len(live_chunks) - 1))
                    rec = outsb.tile([128, 1], f32, tag="rec", name=f"rec{i}")
                    nc.vector.reciprocal(rec[:], op[:, 128:129])
                    osb = outsb.tile([128, 128], bf16, tag="osb", name=f"osb{i}", bufs=3)
                    osb_t[i] = osb
                    nc.vector.tensor_scalar_mul(osb[:], op[:, 0:128], rec[:])

                def stage_T(i):
                    b, h = units[i]
                    otp = psST.tile([128, 128], bf16, tag="sp", name=f"otp{i}")
                    nc.tensor.transpose(otp[:], osb_t[i][:], ident[:])
                    nc.scalar.copy(oT[h][:, b * 128:(b + 1) * 128], otp[:])

                def stage_C(b):
                    for half in range(2):
                        outt = outsb.tile([128, C // 2], bf16, tag="outt",
                                          name=f"outt{b}_{half}", bufs=2)
                        for nn2 in range(2):
                            nn = half * 2 + nn2
                            cp = psC.tile([128, 512], f32, tag="cp", name=f"cp{b}_{nn}")
                            for h in range(HPC):
                                nc.tensor.matmul(
                                    cp[:], oT[h][:, b * 128:(b + 1) * 128],
                                    wo_t[h][:, nn * 512:(nn + 1) * 512],
                                    start=(h == 0), stop=(h == HPC - 1))
                            nc.vector.tensor_copy(outt[:, nn2 * 512:(nn2 + 1) * 512], cp[:])
                        nc.sync.dma_start(
                            o_out[b][:, half * (C // 2):(half + 1) * (C // 2)], outt[:])

                # phase A passes with early attention S-units interleaved
                # so ACT (exp) gets a head start; then the steady pipeline:
                # S runs 4 units ahead of O, T lags O by one, C(b) follows
                # the second head's transpose of each batch.
                qk_pass(0)
                qk_pass(1)
                stage_S(0)
                stage_S(1)
                v_pass(0)
                stage_S(2)
                stage_S(3)
                v_pass(1)
                for i in range(NU):
                    if i + 4 < NU:
                        stage_S(i + 4)
                    stage_O(i)
                    if i >= 1:
                        stage_T(i - 1)
                        b, h = units[i - 1]
                        if h == HPC - 1:
                            stage_C(b)
                stage_T(NU - 1)
                stage_C(units[NU - 1][0])

    nc.compile()
    return nc


def _prep_inputs(x, K_past, V_past, Wq, bq, Wk, bk, Wv, bv, Wo, bo, off):
    """Build the 8 per-core input maps (host-side packing)."""
    proj_dt = BF16 if PROJ_BF16 else np.float32
    x_flat = np.ascontiguousarray(x.reshape(BT, C)).astype(np.float32)
    xt = np.ascontiguousarray(
        x_flat.T.reshape(KC, 128, BT).transpose(1, 0, 2)).reshape(128, KC * BT).astype(proj_dt)

    ident = np.eye(128, dtype=BF16)
    classes = _chunk_classes(off)
    mask_arrs = {c: _mask_tile(off, c) for c in range(NCH) if classes[c] == "part"}

    in_maps = []
    for core in range(NCORES):
        g0 = core * HPC
        im = {"xt": xt, "ident": ident, "identf": np.eye(128, dtype=np.float32)}
        for c, m in mask_arrs.items():
            im[f"mask{c}"] = m
        for h in range(HPC):
            g = g0 + h
            wq_h = Wq[:, g * D:(g + 1) * D]
            im[f"wq{h}"] = np.ascontiguousarray(
                wq_h.reshape(KC, 128, D).transpose(1, 0, 2)).reshape(128, KC * D).astype(proj_dt)
            wk_h = Wk[:, g * D:(g + 1) * D]
            im[f"wk{h}"] = np.ascontiguousarray(
                wk_h.reshape(KC, 128, D).transpose(1, 0, 2)).reshape(128, KC * D).astype(proj_dt)
            im[f"wo{h}"] = np.ascontiguousarray(Wo[g * D:(g + 1) * D, :]).astype(BF16)
            im[f"bq{h}"] = np.ascontiguousarray(bq[g * D:(g + 1) * D]).reshape(128, 1).astype(np.float32)
            im[f"bk{h}"] = np.ascontiguousarray(bk[g * D:(g + 1) * D]).reshape(128, 1).astype(np.float32)
            im[f"bv{h}"] = np.ascontiguousarray(bv[g * D:(g + 1) * D]).reshape(128, 1).astype(np.float32)
        wv_c = Wv[:, g0 * D:(g0 + HPC) * D]  # (2048, 256)
        im["wv"] = np.ascontiguousarray(
            wv_c.reshape(KC, 128, 256).transpose(1, 0, 2)).reshape(128, KC * 256).astype(proj_dt)
        kp_c = K_past[:, g0:g0 + HPC]        # (B, 2, TP, D)
        im["kt"] = np.ascontiguousarray(kp_c.transpose(0, 1, 3, 2)).astype(BF16)
        vp_c = V_past[:, g0:g0 + HPC]
        vp_l = vp_c.reshape(B, HPC, 16, 128, D).transpose(0, 1, 3, 2, 4)  # (B,2,128,16,D)
        vp = np.empty((B, HPC, 128, 16, 129), dtype=BF16)
        vp[:, :, :, :, :D] = vp_l.astype(BF16)
        vp[:, :, :, :, D] = np.array(1.0, BF16)
        im["vp"] = vp.reshape(B, HPC, 128, 16 * 129)
        in_maps.append(im)
    return in_maps


def kernel(x, K_past, V_past, Wq, bq, Wk, bk, Wv, bv, Wo, bo, position_offset):
    global LAST_EXEC_NS, LAST_RESULTS
    off = int(position_offset)
    x = np.asarray(x, dtype=np.float32)
    K_past = np.asarray(K_past, dtype=np.float32)
    V_past = np.asarray(V_past, dtype=np.float32)
    Wq, Wk, Wv, Wo = (np.asarray(a, dtype=np.float32) for a in (Wq, Wk, Wv, Wo))
    bq, bk, bv, bo = (np.asarray(a, dtype=np.float32) for a in (bq, bk, bv, bo))

    cache_key = (off, PROJ_BF16)
    if cache_key not in _PROGRAM_CACHE:
        _PROGRAM_CACHE[cache_key] = _build_program(off)
    nc = _PROGRAM_CACHE[cache_key]

    in_maps = _prep_inputs(x, K_past, V_past, Wq, bq, Wk, bk, Wv, bv, Wo, bo, off)
    res = run_bass_kernel_spmd(nc, in_maps, list(range(NCORES)), trace=TRACE)
    LAST_EXEC_NS = res.exec_time_ns
    LAST_RESULTS = res

    # ---- host assembly ----
    out = np.zeros((BT, C), np.float32)
    for core in range(NCORES):
        out += res.results[core]["out_partial"].reshape(BT, C)
    out += bo[None, :]
    out = out.reshape(B, T, C)

    K_full = np.empty((B, H, TT, D), np.float32)
    V_full = np.empty((B, H, TT, D), np.float32)
    K_full[:, :, :TP] = K_past
    V_full[:, :, :TP] = V_past
    for core in range(NCORES):
        for h in range(HPC):
            g = core * HPC + h
            kn = res.results[core][f"knew{h}"]          # (128 d, BT)
            K_full[:, g, TP:] = kn.reshape(D, B, T).transpose(1, 2, 0)
            vn = res.results[core][f"vnewT{h}"]         # (128 d, BT)
            V_full[:, g, TP:] = vn.reshape(D, B, T).transpose(1, 2, 0)
    return out, K_full, V_full
